# revision 1
# baseline (speedup 1.0000x reference)
"""Causal self-attention (B=4, T=2048, D=1024, H=16) on 8 TRN2 NeuronCores.

Sharding: core c -> (batch b = c//2, head-group g = c%2 of 8 heads).
Each core computes QKV projection for its 8 heads, causal attention, and a
partial out-projection (its heads' rows of W_out). The two partials per batch
are summed on the host during unshard (the "all-reduce after out_proj" of the
tensor-parallel scheme, done host-side since on-device 2-rank collectives are
slower than the host add).

Per-core program (identical SPMD on all 8 cores):
  1. Load x [2048,1024] f32, transpose on TensorE -> xT bf16 [1024(d), 2048(t)]
  2. qT/kT = (Wq|Wk)^T-stationary matmuls -> [512(feat), 2048(t)] bf16
     V     = xT-stationary matmuls -> [2048(t), 512(dv)] bf16, augmented with a
             ones column per head (softmax denominator via the same AV matmul)
  3. Per (head-pair, q-tile of 512): scoresT [k,q] in PSUM (2 heads row-packed
     into the 64x128 PE tiling via partition-half placement), exp on ScalarE
     (scale=1/8, fp32 in -> bf16 out), causal 0/1-mask multiply on diagonal
     tiles (live query sub-ranges only), AV matmuls accumulate [65, 512]
     (64 dv rows + the softmax-denominator row from the ones column),
     normalize via VectorE reciprocal + DRAM-bounce broadcast DMA + VectorE
     multiply -> attnT bf16 [dv, q].
  4. out_proj: attnT-stationary matmuls vs W_out rows -> y partial, DMA out.
     Emitted one q-tile behind the last head-pair's attention so it fills
     TensorE stalls and hides the tail.

Projections for head-pair j+1 are emitted between attention units so the
TensorE fills exp-latency stalls with projection matmuls. Peak engine usage
(cost-model): TensorE ~242us busy of ~297us total; ScalarE ~172us; VectorE
~150us; DMA ~83us.
"""

import numpy as np

import concourse.bass as bass
import concourse.mybir as mybir
import concourse.tile as tile
from concourse.bass_utils import run_bass_kernel_spmd
from concourse.masks import make_identity

F32 = mybir.dt.float32
BF16 = mybir.dt.bfloat16
AX = mybir.AluOpType

T = 2048
D = 1024
HLOC = 8          # heads per core
DKH = 64
QT = 512          # query tile
NQT = T // QT     # 4
KT = 128          # key tile
NDIN = D // 128   # 8
NMT = 4           # q/k feature m-tiles (512 local feats / 128)
VA = 65           # V cols per head incl. ones column
EXP = mybir.ActivationFunctionType.Exp


_NOP_ID = [0]


def _split_multiwaits(nc, limit=1):
    """This toolchain's walrus rejects more than one sync-wait on an
    instruction ("Too many sync wait commands"). Move excess waits onto
    same-engine NOPs inserted immediately before the instruction — the
    engine sequencer executes them in program order, so semantics are
    preserved (issue-after-wait implies execute-after-wait for DMA too)."""
    for f in nc.m.functions:
        for blk in f.blocks:
            new = []
            changed = False
            for inst in blk.instructions:
                si = inst.sync_info
                if si is not None and len(si.on_wait) > limit:
                    waits = list(si.on_wait)
                    inst.sync_info = mybir.SyncInfo(
                        on_wait=waits[:limit], on_update=list(si.on_update))
                    for w in waits[limit:]:
                        _NOP_ID[0] += 1
                        nop = mybir.InstNoOp(
                            name=f"waitnop-{_NOP_ID[0]}", ins=[], outs=[])
                        nop.engine = inst.engine
                        nop.sync_info = mybir.SyncInfo(on_wait=[w], on_update=[])
                        new.append(nop)
                    changed = True
                new.append(inst)
            if changed:
                blk.instructions = new


def build_nc():
    nc = bass.Bass()
    x_ext = nc.declare_dram_parameter("x", [T, D], F32, isOutput=False)
    w_ext = nc.declare_dram_parameter("W_qkv", [D, 3 * 512], F32, isOutput=False)
    b_ext = nc.declare_dram_parameter("b_qkv", [3 * 512], F32, isOutput=False)
    wo_ext = nc.declare_dram_parameter("W_out", [512, D], F32, isOutput=False)
    out_ext = nc.declare_dram_parameter("out", [T, D], F32, isOutput=True)

    with tile.TileContext(nc) as tc:
        with (
            tc.tile_pool(name="const", bufs=1) as constp,
            tc.tile_pool(name="big", bufs=1) as bigp,
        ):
            ident = constp.tile([128, 128], F32, tag="ident")
            make_identity(nc, ident)

            # causal 0/1 mask [128, 2x512] bf16 (head-duplicated triangle):
            # mask[p, (h, f)] = 1 if f >= p else 0 — diagonal k-tiles reduce to
            # this one pattern after live-range slicing.
            maskt = constp.tile([128, 1024], BF16, tag="maskt")
            nc.gpsimd.memset(maskt, 1.0)
            mk3 = maskt.rearrange("p (h f) -> p h f", f=512)
            nc.gpsimd.affine_select(
                out=mk3, in_=mk3,
                compare_op=AX.is_ge, fill=0.0,
                base=0, channel_multiplier=-1,
                pattern=[[0, 2], [1, 512]],
            )

            # biases: per-partition vectors for q/k feature tiles, broadcast
            # tile for V (bias along the free dv axis)
            bq_sb = constp.tile([128, NMT], F32, tag="bq")
            bk_sb = constp.tile([128, NMT], F32, tag="bk")
            nc.sync.dma_start(
                out=bq_sb, in_=b_ext[0:512].rearrange("(m p) -> p m", p=128))
            nc.sync.dma_start(
                out=bk_sb, in_=b_ext[512:1024].rearrange("(m p) -> p m", p=128))
            bv_sb = constp.tile([128, 512], F32, tag="bv")
            bv_src = b_ext[1024:1536]
            nc.sync.dma_start(
                out=bv_sb,
                in_=bass.AP(tensor=bv_src.tensor, offset=bv_src.offset,
                            ap=[[0, 128]] + list(bv_src.ap)),
            )

            # persistent activations
            qT = bigp.tile([128, NMT * T], BF16, tag="qT")
            kT = bigp.tile([128, NMT * T], BF16, tag="kT")
            vaug = bigp.tile([128, (T // 128) * HLOC * VA], BF16, tag="vaug")
            attnT = bigp.tile([128, NMT * T], BF16, tag="attnT")
            woutb = bigp.tile([128, 4 * D], BF16, tag="woutb")

            # ------- phases B+C interleaved: transpose x, project, attention
            with (
                tc.tile_pool(name="proj_sb", bufs=1) as projp,
                tc.tile_pool(name="pjpsum", bufs=2, space="PSUM") as pjpsum,
            ):
                xT = projp.tile([128, NDIN * T], BF16, tag="xT")
                wbf = projp.tile([128, NDIN * 1536], BF16, tag="wbf")

                # x load + TensorE transpose (f32) + cast to bf16 on DVE;
                # staging pools close before the attention pools open
                with (
                    tc.tile_pool(name="xstage", bufs=5) as xstage,
                    tc.tile_pool(name="wstage", bufs=3) as wstage,
                    tc.tile_pool(name="tpsum", bufs=2, space="PSUM") as tpsum,
                ):
                    for tt in range(T // 128):
                        xt = xstage.tile([128, D], F32, tag="x")
                        nc.sync.dma_start(
                            out=xt, in_=x_ext[tt * 128:(tt + 1) * 128, :])
                        for g in range(2):
                            tp = tpsum.tile([128, 512], F32, tag="tp")
                            for j in range(4):
                                dj = g * 4 + j
                                nc.tensor.transpose(
                                    tp[:, j * 128:(j + 1) * 128],
                                    xt[:, dj * 128:(dj + 1) * 128], ident)
                            xT_dst = bass.AP(
                                tensor=xT.tensor,
                                offset=xT.offset + (g * 4) * T + tt * 128,
                                ap=[list(xT.ap[0]), [T, 4], [1, 128]])
                            nc.vector.tensor_copy(
                                xT_dst,
                                tp.rearrange("p (j t) -> p j t", t=128))

                    # weight loads + casts (ScalarE is idle this early)
                    for k in range(4):
                        wot = wstage.tile([128, D], F32, tag="wo")
                        nc.sync.dma_start(out=wot, in_=wo_ext[k * 128:(k + 1) * 128, :])
                        nc.scalar.copy(woutb[:, k * D:(k + 1) * D], wot)
                    for k in range(NDIN):
                        wt = wstage.tile([128, 1536], F32, tag="wqkv")
                        nc.sync.dma_start(out=wt, in_=w_ext[k * 128:(k + 1) * 128, :])
                        nc.scalar.copy(wbf[:, k * 1536:(k + 1) * 1536], wt)

                def emit_qkproj(mt, ns=None):
                    for sec, dst, bias in ((0, qT, bq_sb), (1, kT, bk_sb)):
                        for n in (range(NQT) if ns is None else ns):
                            ps = pjpsum.tile([128, 512], F32, tag="pj")
                            for kk in range(NDIN):
                                nc.tensor.matmul(
                                    ps,
                                    lhsT=wbf[:, kk * 1536 + sec * 512 + mt * 128:
                                             kk * 1536 + sec * 512 + (mt + 1) * 128],
                                    rhs=xT[:, kk * T + n * 512: kk * T + (n + 1) * 512],
                                    start=(kk == 0), stop=(kk == NDIN - 1),
                                )
                            nc.scalar.activation(
                                dst[:, mt * T + n * 512: mt * T + (n + 1) * 512],
                                ps, mybir.ActivationFunctionType.Identity,
                                bias=bias[:, mt:mt + 1], scale=1.0)

                def emit_vproj(tts):
                    for tt in tts:
                        ps = pjpsum.tile([128, 512], F32, tag="pj")
                        for kk in range(NDIN):
                            nc.tensor.matmul(
                                ps,
                                lhsT=xT[:, kk * T + tt * 128: kk * T + (tt + 1) * 128],
                                rhs=wbf[:, kk * 1536 + 1024: kk * 1536 + 1536],
                                start=(kk == 0), stop=(kk == NDIN - 1),
                            )
                        blk = vaug[:, tt * (HLOC * VA):(tt + 1) * (HLOC * VA)]
                        blk3 = blk.rearrange("p (h c) -> p h c", c=VA)
                        nc.vector.tensor_tensor(
                            out=blk3[:, :, 0:64],
                            in0=ps.rearrange("p (h c) -> p h c", c=64),
                            in1=bv_sb.rearrange("p (h c) -> p h c", c=64),
                            op=AX.add)
                        nc.vector.memset(blk3[:, :, 64:65], 1.0)

                with (
                    tc.tile_pool(name="scps", bufs=2, space="PSUM") as scps,
                    tc.tile_pool(name="avps", bufs=2, space="PSUM") as avps,
                    tc.tile_pool(name="ptp", bufs=22) as ptp,
                    tc.tile_pool(name="accp", bufs=3) as accp,
                    tc.tile_pool(name="recp", bufs=2) as recp,
                    tc.tile_pool(name="dscr", bufs=4, space="DRAM") as dscr,
                    tc.tile_pool(name="yo", bufs=3) as yo,
                ):
                    def emit_unit(hp, qt):
                        nkt = 4 * (qt + 1)
                        accs = [avps.tile([128, 512], F32, tag="av",
                                          name=f"av{par}")
                                for par in range(2)]
                        m3 = maskt.rearrange("p (h q) -> p h q", q=512)
                        for c0 in range(0, nkt, 8):
                            chunk = list(range(c0, min(c0 + 8, nkt)))
                            pts = {}
                            for kt in chunk:
                                # diagonal k-tile i (k0 = q0+128i): only queries
                                # f >= 128i are live; compute/exp/mask that range
                                i = kt - (nkt - 4)
                                lo_q = max(0, 128 * i)
                                ps = scps.tile([128, 1024], F32, tag="sc")
                                for par in range(2):
                                    lo, hi = par * 64, par * 64 + 64
                                    nc.tensor.matmul(
                                        ps[:, par * 512 + lo_q:(par + 1) * 512],
                                        lhsT=kT[lo:hi, hp * T + kt * 128:
                                                hp * T + (kt + 1) * 128],
                                        rhs=qT[lo:hi, hp * T + qt * 512 + lo_q:
                                               hp * T + (qt + 1) * 512],
                                        start=True, stop=True,
                                    )
                                pt_t = ptp.tile([128, 1024], BF16, tag="pt")
                                ps3 = ps.rearrange("p (h q) -> p h q", q=512)
                                pt3 = pt_t.rearrange("p (h q) -> p h q", q=512)
                                nc.scalar.activation(
                                    pt3[:, :, lo_q:512], ps3[:, :, lo_q:512],
                                    EXP, bias=0.0, scale=0.125)
                                if i >= 0:
                                    nc.vector.tensor_tensor(
                                        out=pt3[:, :, lo_q:512],
                                        in0=pt3[:, :, lo_q:512],
                                        in1=m3[:, :, 0:512 - lo_q], op=AX.mult)
                                pts[kt] = (pt_t, lo_q)
                            for kt in chunk:
                                for par in range(2):
                                    h = 2 * hp + par
                                    pt_t, lo_q = pts[kt]
                                    nc.tensor.matmul(
                                        accs[par][0:VA, lo_q:512],
                                        lhsT=vaug[:, kt * (HLOC * VA) + h * VA:
                                                  kt * (HLOC * VA) + (h + 1) * VA],
                                        rhs=pt_t[:, par * 512 + lo_q:(par + 1) * 512],
                                        start=(kt == 0), stop=(kt == nkt - 1),
                                    )
                        for par in range(2):
                            acc = accs[par]
                            accsb = accp.tile([VA, 512], F32, tag="accs")
                            nc.vector.tensor_copy(accsb, acc[0:VA, :])
                            rc = recp.tile([1, 512], F32, tag="rc")
                            rb = recp.tile([64, 512], F32, tag="rb")
                            nc.vector.reciprocal(rc, accsb[64:65, :])
                            rd = dscr.tile([1, 512], F32, tag="rd")
                            nc.sync.dma_start(out=rd, in_=rc)
                            nc.sync.dma_start(
                                out=rb,
                                in_=bass.AP(tensor=rd.tensor, offset=rd.offset,
                                            ap=[[0, 64]] + list(rd.ap)[1:]))
                            nc.vector.tensor_tensor(
                                out=attnT[par * 64:(par + 1) * 64,
                                          hp * T + qt * 512: hp * T + (qt + 1) * 512],
                                in0=accsb[0:64, :], in1=rb, op=AX.mult)

                    def emit_outproj(mts):
                        for mt in mts:
                            for n in range(2):
                                ps = pjpsum.tile([128, 512], F32, tag="pj",
                                                 name="y")
                                for kk in range(4):
                                    nc.tensor.matmul(
                                        ps,
                                        lhsT=attnT[:, kk * T + mt * 128:
                                                   kk * T + (mt + 1) * 128],
                                        rhs=woutb[:, kk * D + n * 512:
                                                  kk * D + (n + 1) * 512],
                                        start=(kk == 0), stop=(kk == 3))
                                yt = yo.tile([128, 512], F32, tag="yt",
                                             name="yt")
                                nc.vector.tensor_copy(yt, ps)
                                nc.sync.dma_start(
                                    out=out_ext[mt * 128:(mt + 1) * 128,
                                                n * 512:(n + 1) * 512], in_=yt)

                    for hp in range(HLOC // 2):
                        if hp >= 1:
                            emit_qkproj(hp)
                        for qt in range(NQT):
                            if hp == 0:
                                emit_qkproj(0, ns=[qt])
                                emit_vproj(range(4 * qt, 4 * qt + 4))
                            emit_unit(hp, qt)
                            if hp == 3 and qt >= 1:
                                emit_outproj(range(4 * (qt - 1), 4 * qt))
                    emit_outproj(range(12, 16))

    _split_multiwaits(nc)
    return nc


_NC_CACHE = {}


def get_nc():
    if "nc" not in _NC_CACHE:
        _NC_CACHE["nc"] = build_nc()
    return _NC_CACHE["nc"]


def make_in_maps(x, W_qkv, b_qkv, W_out):
    in_maps = []
    for c in range(8):
        b, g = c // 2, c % 2
        s = slice(512 * g, 512 * (g + 1))
        wslice = np.concatenate(
            [W_qkv[:, 512 * g:512 * (g + 1)],
             W_qkv[:, 1024 + 512 * g:1024 + 512 * (g + 1)],
             W_qkv[:, 2048 + 512 * g:2048 + 512 * (g + 1)]], axis=1)
        bslice = np.concatenate(
            [b_qkv[512 * g:512 * (g + 1)],
             b_qkv[1024 + 512 * g:1024 + 512 * (g + 1)],
             b_qkv[2048 + 512 * g:2048 + 512 * (g + 1)]])
        in_maps.append({
            "x": np.ascontiguousarray(x[b], dtype=np.float32),
            "W_qkv": np.ascontiguousarray(wslice, dtype=np.float32),
            "b_qkv": np.ascontiguousarray(bslice, dtype=np.float32),
            "W_out": np.ascontiguousarray(W_out[s], dtype=np.float32),
        })
    return in_maps


def kernel(x, W_qkv, b_qkv, W_out, b_out):
    x = np.asarray(x)
    W_qkv = np.asarray(W_qkv)
    b_qkv = np.asarray(b_qkv)
    W_out = np.asarray(W_out)
    b_out = np.asarray(b_out)
    nc = get_nc()
    in_maps = make_in_maps(x, W_qkv, b_qkv, W_out)
    res = run_bass_kernel_spmd(nc, in_maps, core_ids=list(range(8))).results
    out = np.stack(
        [res[2 * b]["out"] + res[2 * b + 1]["out"] for b in range(4)], axis=0)
    out = out + b_out[None, None, :]
    return out.astype(np.float32)



# revision 13
# speedup vs baseline: 1.0866x; 1.0866x over previous
"""Causal self-attention (B=4, T=2048, D=1024, H=16) on 8 TRN2 NeuronCores.

Sharding: core c -> (batch b = c//2, head-group g = c%2 of 8 heads).
Each core computes QKV projection for its 8 heads, causal attention, and a
partial out-projection (its heads' rows of W_out). The two partials per batch
are summed on the host during unshard (the "all-reduce after out_proj" of the
tensor-parallel scheme).

Per-core program (identical SPMD on all 8 cores):
  1. Loads: gpsimd (SWDGE) casting DMAs bring x, W_qkv, W_out from DRAM f32
     into SBUF as bf16 directly; xT [1024(d), 2048(t)] is produced by DMA
     xbar-transposes (16x128 tiles) issued on the ACT queue. No PE transposes
     and no engine-side casts anywhere.
  2. qT/kT = W^T-stationary matmuls -> [512(feat), 2048(t)] bf16 with the
     bias added during the PSUM->SBUF move on VectorE; V = xT-stationary
     matmuls -> [2048(t), 8 heads x 65] bf16 augmented with a ones column
     per head (softmax denominator via the same AV matmul).
  3. Per (head-pair, q-tile of 512): scoresT [k,q] in PSUM (2 heads packed
     into partition halves of the PE), exp on ScalarE (scale=1/8, fp32 in ->
     bf16 out, live query sub-ranges only), causal 0/1-mask multiply on the
     four diagonal 128x128 blocks only (VectorE). AV runs in the flipped
     orientation out[q(128 part), 65(free)] = P[k,qsub]^T @ Vh[k,65], one
     PSUM-accumulated chain per (q-subtile, head) packed 4-per-bank in a
     single 2-bank accumulator tile; dead (k>q) subtiles are skipped
     entirely. This costs 65 PE rows per (ktile, head, qsub) instead of the
     512 of the [65 part, q free] orientation. Normalization is a batched
     VectorE reciprocal of the 8 denominator columns plus two stride-0
     broadcast multiplies into attn [t, dloc] bf16.
  4. attn chunks are DMA-xbar-transposed into attnT [dloc, t]; out_proj
     matmuls (attnT-stationary vs W_out rows) write PSUM which is DMA'd
     straight to DRAM. Emitted one q-tile behind the last head-pair.

Projections for head-pair j+1 are emitted between attention units so the
TensorE fills exp-latency stalls with projection matmuls.
"""

import numpy as np

import concourse.bass as bass
import concourse.mybir as mybir
import concourse.tile as tile
from concourse.bass_utils import run_bass_kernel_spmd

DEBUG_DUMP = False

F32 = mybir.dt.float32
BF16 = mybir.dt.bfloat16
AX = mybir.AluOpType

T = 2048
D = 1024
HLOC = 8          # heads per core
DKH = 64
QT = 512          # query tile
NQT = T // QT     # 4
NDIN = D // 128   # 8
NMT = 4           # q/k feature m-tiles (512 local feats / 128)
VA = 65           # V cols per head incl. ones column
NTT = T // 128    # 16
EXP = mybir.ActivationFunctionType.Exp


_NOP_ID = [0]


def _split_multiwaits(nc, limit=1):
    """This toolchain's walrus rejects more than one sync-wait on an
    instruction ("Too many sync wait commands"), and rejects any sync-wait
    on the DMA-xpose descriptor. Move excess waits onto same-engine NOPs
    inserted immediately before the instruction — the engine sequencer
    executes them in program order, so semantics are preserved."""
    for f in nc.m.functions:
        for blk in f.blocks:
            new = []
            changed = False
            for inst in blk.instructions:
                si = inst.sync_info
                lim = 0 if type(inst).__name__ == "InstDmaTransposeAnt" else limit
                if si is not None and len(si.on_wait) > lim:
                    waits = list(si.on_wait)
                    inst.sync_info = mybir.SyncInfo(
                        on_wait=waits[:lim], on_update=list(si.on_update))
                    for w in waits[lim:]:
                        _NOP_ID[0] += 1
                        nop = mybir.InstNoOp(
                            name=f"waitnop-{_NOP_ID[0]}", ins=[], outs=[])
                        nop.engine = inst.engine
                        nop.sync_info = mybir.SyncInfo(on_wait=[w], on_update=[])
                        new.append(nop)
                    changed = True
                new.append(inst)
            if changed:
                blk.instructions = new


def build_nc():
    nc = bass.Bass()
    x_ext = nc.declare_dram_parameter("x", [T, D], F32, isOutput=False)
    w_ext = nc.declare_dram_parameter("W_qkv", [D, 3 * 512], F32, isOutput=False)
    b_ext = nc.declare_dram_parameter("b_qkv", [3 * 512], F32, isOutput=False)
    wo_ext = nc.declare_dram_parameter("W_out", [512, D], F32, isOutput=False)
    out_ext = nc.declare_dram_parameter("out", [T, D], F32, isOutput=True)
    if DEBUG_DUMP:
        dbg = {
            "xT": nc.declare_dram_parameter("d_xT", [128, NDIN * T], F32, isOutput=True),
            "qT": nc.declare_dram_parameter("d_qT", [128, NMT * T], F32, isOutput=True),
            "kT": nc.declare_dram_parameter("d_kT", [128, NMT * T], F32, isOutput=True),
            "vaug": nc.declare_dram_parameter("d_vaug", [128, NTT * HLOC * VA], F32, isOutput=True),
            "attn": nc.declare_dram_parameter("d_attn", [128, NTT * 512], F32, isOutput=True),
            "attnT": nc.declare_dram_parameter("d_attnT", [128, 4 * T], F32, isOutput=True),
        }

    with tile.TileContext(nc) as tc:
        with (
            tc.tile_pool(name="const", bufs=1) as constp,
            tc.tile_pool(name="big", bufs=1) as bigp,
        ):
            # causal 0/1 mask [128, 2x512] bf16 (head-duplicated triangle):
            # mask[p, (h, f)] = 1 if f >= p else 0. Diagonal 128x128 blocks
            # use the [:, :, 0:128] slice.
            maskt = constp.tile([128, 1024], BF16, tag="maskt")
            nc.gpsimd.memset(maskt, 1.0)
            mk3 = maskt.rearrange("p (h f) -> p h f", f=512)
            nc.gpsimd.affine_select(
                out=mk3, in_=mk3,
                compare_op=AX.is_ge, fill=0.0,
                base=0, channel_multiplier=-1,
                pattern=[[0, 2], [1, 512]],
            )

            # biases: per-partition vectors for q/k feature tiles, broadcast
            # tile for V (bias along the free dv axis)
            bq_sb = constp.tile([128, NMT], F32, tag="bq")
            bk_sb = constp.tile([128, NMT], F32, tag="bk")
            nc.sync.dma_start(
                out=bq_sb, in_=b_ext[0:512].rearrange("(m p) -> p m", p=128))
            nc.sync.dma_start(
                out=bk_sb, in_=b_ext[512:1024].rearrange("(m p) -> p m", p=128))
            bv_sb = constp.tile([128, 512], F32, tag="bv")
            bv_src = b_ext[1024:1536]
            nc.sync.dma_start(
                out=bv_sb,
                in_=bass.AP(tensor=bv_src.tensor, offset=bv_src.offset,
                            ap=[[0, 128]] + list(bv_src.ap)),
            )

            # persistent activations / weights (all bf16)
            qT = bigp.tile([128, NMT * T], BF16, tag="qT")
            kT = bigp.tile([128, NMT * T], BF16, tag="kT")
            vaug = bigp.tile([128, NTT * HLOC * VA], BF16, tag="vaug")
            attn = bigp.tile([128, NTT * 512], BF16, tag="attn")
            attnT = bigp.tile([128, 4 * T], BF16, tag="attnT")
            woutb = bigp.tile([128, 4 * D], BF16, tag="woutb")
            wbf = bigp.tile([128, NDIN * 1536], BF16, tag="wbf")
            xT = bigp.tile([128, NDIN * T], BF16, tag="xT")

            with (
                tc.tile_pool(name="pjpsum", bufs=2, space="PSUM") as pjpsum,
            ):
                # ---- loads: gpsimd casting DMAs (f32->bf16 in flight,
                # batched 4 row-chunks per DMA to amortize SWDGE gen) +
                # ACT-issued DMA xbar-transposes for xT
                def cast_load(dst, dst_off, src, row0, nrow, width):
                    nc.gpsimd.dma_start(
                        out=bass.AP(
                            tensor=dst.tensor, offset=dst.offset + dst_off,
                            ap=[list(dst.ap[0]), [width, nrow], [1, width]]),
                        in_=bass.AP(
                            tensor=src.tensor,
                            offset=src.offset + row0 * width,
                            ap=[[width, 128], [128 * width, nrow], [1, width]]),
                    )

                # 2 row-chunks (256 descriptors) per DMA so 4 fit in the
                # SWDGE descriptor ring; x group 0 first (the xT transposes
                # are on the critical path), weights interleaved behind it.
                # x: SP f32 loads -> PE transpose (f32) -> DVE cast to bf16.
                # W: Pool casting DMAs (f32->bf16 in flight), interleaved so
                # the first W chunks land before the first projection chains.
                from concourse.masks import make_identity
                ident = constp.tile([128, 128], F32, tag="ident")
                make_identity(nc, ident)
                with (
                    tc.tile_pool(name="xstage", bufs=6) as xstage,
                    tc.tile_pool(name="tpsum", bufs=2, space="PSUM") as tpsum,
                ):
                    def load_tt(tt):
                        xt = xstage.tile([128, D], F32, tag="x")
                        nc.sync.dma_start(
                            out=xt, in_=x_ext[tt * 128:(tt + 1) * 128, :])
                        for g in range(2):
                            tp = tpsum.tile([128, 512], F32, tag="tp")
                            for j in range(4):
                                dj = g * 4 + j
                                nc.tensor.transpose(
                                    tp[:, j * 128:(j + 1) * 128],
                                    xt[:, dj * 128:(dj + 1) * 128], ident)
                            xT_dst = bass.AP(
                                tensor=xT.tensor,
                                offset=xT.offset + (g * 4) * T + tt * 128,
                                ap=[list(xT.ap[0]), [T, 4], [1, 128]])
                            nc.vector.tensor_copy(
                                xT_dst,
                                tp.rearrange("p (j t) -> p j t", t=128))

                    for tt in range(4):
                        load_tt(tt)
                    cast_load(wbf, 0, w_ext[0:1, :], 0, 4, 1536)
                    for tt in range(4, 8):
                        load_tt(tt)
                    cast_load(wbf, 4 * 1536, w_ext[0:1, :], 512, 4, 1536)
                    for tt in range(8, 12):
                        load_tt(tt)
                    cast_load(woutb, 0, wo_ext[0:1, :], 0, 4, D)
                    for tt in range(12, NTT):
                        load_tt(tt)

                def emit_qkproj(mt, ns=None):
                    for sec, dst, bias in ((0, qT, bq_sb), (1, kT, bk_sb)):
                        for n in (range(NQT) if ns is None else ns):
                            ps = pjpsum.tile([128, 512], F32, tag="pj")
                            for kk in range(NDIN):
                                nc.tensor.matmul(
                                    ps,
                                    lhsT=wbf[:, kk * 1536 + sec * 512 + mt * 128:
                                             kk * 1536 + sec * 512 + (mt + 1) * 128],
                                    rhs=xT[:, kk * T + n * 512: kk * T + (n + 1) * 512],
                                    start=(kk == 0), stop=(kk == NDIN - 1),
                                )
                            nc.vector.tensor_scalar(
                                out=dst[:, mt * T + n * 512: mt * T + (n + 1) * 512],
                                in0=ps, scalar1=bias[:, mt:mt + 1], scalar2=None,
                                op0=AX.add)

                def emit_vproj(tts):
                    for tt in tts:
                        ps = pjpsum.tile([128, 512], F32, tag="pj")
                        for kk in range(NDIN):
                            nc.tensor.matmul(
                                ps,
                                lhsT=xT[:, kk * T + tt * 128: kk * T + (tt + 1) * 128],
                                rhs=wbf[:, kk * 1536 + 1024: kk * 1536 + 1536],
                                start=(kk == 0), stop=(kk == NDIN - 1),
                            )
                        blk = vaug[:, tt * (HLOC * VA):(tt + 1) * (HLOC * VA)]
                        blk3 = blk.rearrange("p (h c) -> p h c", c=VA)
                        nc.vector.tensor_tensor(
                            out=blk3[:, :, 0:64],
                            in0=ps.rearrange("p (h c) -> p h c", c=64),
                            in1=bv_sb.rearrange("p (h c) -> p h c", c=64),
                            op=AX.add)
                        nc.vector.memset(blk3[:, :, 64:65], 1.0)

                with (
                    tc.tile_pool(name="scps", bufs=2, space="PSUM") as scps,
                    tc.tile_pool(name="avps", bufs=1, space="PSUM") as avps,
                    tc.tile_pool(name="ptp", bufs=10) as ptp,
                    tc.tile_pool(name="recp", bufs=2) as recp,
                    tc.tile_pool(name="yo", bufs=3) as yo,
                ):
                    m3 = maskt.rearrange("p (h q) -> p h q", q=512)

                    def av_off(j, par):
                        s = 2 * j + par
                        return (s // 4) * 512 + (s % 4) * 65

                    def emit_unit(hp, qt):
                        nkt = 4 * (qt + 1)
                        avt = avps.tile([128, 1024], F32, tag="av")
                        for c0 in range(0, nkt, 8):
                            chunk = list(range(c0, min(c0 + 8, nkt)))
                            pts = {}
                            for kt in chunk:
                                # diagonal k-tile (i >= 0): only queries
                                # f >= 128*i are live
                                i = kt - (nkt - 4)
                                lo_q = max(0, 128 * i)
                                ps = scps.tile([128, 1024], F32, tag="sc")
                                for par in range(2):
                                    lo, hi = par * 64, par * 64 + 64
                                    nc.tensor.matmul(
                                        ps[:, par * 512 + lo_q:(par + 1) * 512],
                                        lhsT=kT[lo:hi, hp * T + kt * 128:
                                                hp * T + (kt + 1) * 128],
                                        rhs=qT[lo:hi, hp * T + qt * 512 + lo_q:
                                               hp * T + (qt + 1) * 512],
                                        start=True, stop=True,
                                    )
                                pt_t = ptp.tile([128, 1024], BF16, tag="pt")
                                ps3 = ps.rearrange("p (h q) -> p h q", q=512)
                                pt3 = pt_t.rearrange("p (h q) -> p h q", q=512)
                                nc.scalar.activation(
                                    pt3[:, :, lo_q:512], ps3[:, :, lo_q:512],
                                    EXP, bias=0.0, scale=0.125)
                                if i >= 0:
                                    # mask the diagonal 128x128 block (both
                                    # heads): q-subtile j == i
                                    blk = bass.AP(
                                        tensor=pt_t.tensor,
                                        offset=pt_t.offset + lo_q,
                                        ap=[list(pt_t.ap[0]), [512, 2], [1, 128]])
                                    mblk = bass.AP(
                                        tensor=maskt.tensor,
                                        offset=maskt.offset,
                                        ap=[list(maskt.ap[0]), [512, 2], [1, 128]])
                                    nc.vector.tensor_tensor(
                                        out=blk, in0=blk, in1=mblk, op=AX.mult)
                                pts[kt] = pt_t
                            for kt in chunk:
                                pt_t = pts[kt]
                                for j in range(4):
                                    if kt > 4 * qt + j:
                                        continue  # fully-dead block
                                    for par in range(2):
                                        h = 2 * hp + par
                                        off = av_off(j, par)
                                        # PSUM start/stop are per 2KB bank
                                        # (zero region): exactly one start
                                        # (zeroes the bank) and one stop per
                                        # bank; sibling chains ride on it.
                                        nc.tensor.matmul(
                                            avt[:, off:off + VA],
                                            lhsT=pt_t[:, par * 512 + j * 128:
                                                      par * 512 + (j + 1) * 128],
                                            rhs=vaug[:, kt * (HLOC * VA) + h * VA:
                                                     kt * (HLOC * VA) + (h + 1) * VA],
                                            start=(kt == 0 and par == 0
                                                   and j % 2 == 0),
                                            stop=(par == 1 and j % 2 == 1
                                                  and kt == 4 * qt + j),
                                        )
                        # normalize: rc[p, s] = 1 / denom(slot s); attn chunk
                        # cols (hp,par) of t-chunks 4qt..4qt+3
                        rc = recp.tile([128, 8], F32, tag="rc")
                        nc.vector.reciprocal(
                            rc,
                            bass.AP(tensor=avt.tensor, offset=avt.offset + 64,
                                    ap=[list(avt.ap[0]), [512, 2], [130, 2],
                                        [65, 2]]))
                        for par in range(2):
                            dst = bass.AP(
                                tensor=attn.tensor,
                                offset=attn.offset + (4 * qt) * 512
                                + (hp * 2 + par) * 64,
                                ap=[list(attn.ap[0]), [1024, 2], [512, 2],
                                    [1, 64]])
                            src = bass.AP(
                                tensor=avt.tensor,
                                offset=avt.offset + par * 65,
                                ap=[list(avt.ap[0]), [512, 2], [130, 2],
                                    [1, 64]])
                            rcb = bass.AP(
                                tensor=rc.tensor, offset=rc.offset + par,
                                ap=[list(rc.ap[0]), [4, 2], [2, 2], [0, 64]])
                            nc.vector.tensor_tensor(
                                out=dst, in0=src, in1=rcb, op=AX.mult)

                    def emit_attnT(qt):
                        for j in range(4):
                            tt = 4 * qt + j
                            nc.sync.dma_start_transpose(
                                out=bass.AP(
                                    tensor=attnT.tensor,
                                    offset=attnT.offset + tt * 128,
                                    ap=[list(attnT.ap[0]), [T, 4], [1, 128]]),
                                in_=attn[:, tt * 512:(tt + 1) * 512])

                    def emit_outproj(mts):
                        for mt in mts:
                            for n in range(2):
                                ps = pjpsum.tile([128, 512], F32, tag="pj",
                                                 name="y")
                                for kk in range(4):
                                    nc.tensor.matmul(
                                        ps,
                                        lhsT=attnT[:, kk * T + mt * 128:
                                                   kk * T + (mt + 1) * 128],
                                        rhs=woutb[:, kk * D + n * 512:
                                                  kk * D + (n + 1) * 512],
                                        start=(kk == 0), stop=(kk == 3))
                                yt = yo.tile([128, 512], F32, tag="yt",
                                             name="yt")
                                nc.vector.tensor_copy(yt, ps)
                                nc.sync.dma_start(
                                    out=out_ext[mt * 128:(mt + 1) * 128,
                                                n * 512:(n + 1) * 512], in_=yt)

                    for hp in range(HLOC // 2):
                        if hp >= 1:
                            emit_qkproj(hp)
                        for qt in range(NQT):
                            if hp == 0:
                                emit_qkproj(0, ns=[qt])
                                emit_vproj(range(4 * qt, 4 * qt + 4))
                            emit_unit(hp, qt)
                            if hp == 3:
                                emit_attnT(qt)
                                if qt >= 1:
                                    emit_outproj(range(4 * (qt - 1), 4 * qt))
                    emit_outproj(range(12, 16))
                    if DEBUG_DUMP:
                        with tc.tile_pool(name="dbgp", bufs=2) as dbgp:
                            for nm, src in (("xT", xT), ("qT", qT), ("kT", kT),
                                            ("vaug", vaug), ("attn", attn),
                                            ("attnT", attnT)):
                                w = src.shape[1]
                                for c0 in range(0, w, 2048):
                                    cw = min(2048, w - c0)
                                    t = dbgp.tile([128, cw], F32, tag="dbg")
                                    nc.vector.tensor_copy(t, src[:, c0:c0 + cw])
                                    nc.sync.dma_start(
                                        out=dbg[nm][0:128, c0:c0 + cw], in_=t)

    _split_multiwaits(nc)
    return nc


_NC_CACHE = {}


def get_nc():
    if "nc" not in _NC_CACHE:
        _NC_CACHE["nc"] = build_nc()
    return _NC_CACHE["nc"]


def make_in_maps(x, W_qkv, b_qkv, W_out):
    in_maps = []
    for c in range(8):
        b, g = c // 2, c % 2
        s = slice(512 * g, 512 * (g + 1))
        wslice = np.concatenate(
            [W_qkv[:, 512 * g:512 * (g + 1)],
             W_qkv[:, 1024 + 512 * g:1024 + 512 * (g + 1)],
             W_qkv[:, 2048 + 512 * g:2048 + 512 * (g + 1)]], axis=1)
        bslice = np.concatenate(
            [b_qkv[512 * g:512 * (g + 1)],
             b_qkv[1024 + 512 * g:1024 + 512 * (g + 1)],
             b_qkv[2048 + 512 * g:2048 + 512 * (g + 1)]])
        in_maps.append({
            "x": np.ascontiguousarray(x[b], dtype=np.float32),
            "W_qkv": np.ascontiguousarray(wslice, dtype=np.float32),
            "b_qkv": np.ascontiguousarray(bslice, dtype=np.float32),
            "W_out": np.ascontiguousarray(W_out[s], dtype=np.float32),
        })
    return in_maps


def kernel(x, W_qkv, b_qkv, W_out, b_out):
    x = np.asarray(x)
    W_qkv = np.asarray(W_qkv)
    b_qkv = np.asarray(b_qkv)
    W_out = np.asarray(W_out)
    b_out = np.asarray(b_out)
    nc = get_nc()
    in_maps = make_in_maps(x, W_qkv, b_qkv, W_out)
    res = run_bass_kernel_spmd(nc, in_maps, core_ids=list(range(8))).results
    out = np.stack(
        [res[2 * b]["out"] + res[2 * b + 1]["out"] for b in range(4)], axis=0)
    out = out + b_out[None, None, :]
    return out.astype(np.float32)


# revision 36
# speedup vs baseline: 1.1793x; 1.0853x over previous
"""Causal self-attention (B=4, T=2048, D=1024, H=16) on 8 TRN2 NeuronCores.

Sharding: core c -> (batch b = c//2, head-group g = c%2 of 8 heads).
Each core computes QKV projection for its 8 heads, causal attention, and a
partial out-projection (its heads' rows of W_out). The two partials per batch
are summed on the host during unshard (the "all-reduce after out_proj" of the
tensor-parallel scheme).

Per-core program (identical SPMD on all 8 cores):
  1. Loads: gpsimd (SWDGE) casting DMAs bring x, W_qkv, W_out from DRAM f32
     into SBUF as bf16 directly; xT [1024(d), 2048(t)] is produced by DMA
     xbar-transposes (16x128 tiles) issued on the ACT queue. No PE transposes
     and no engine-side casts anywhere.
  2. qT/kT = W^T-stationary matmuls -> [512(feat), 2048(t)] bf16 with the
     bias added during the PSUM->SBUF move on VectorE; V = xT-stationary
     matmuls -> [2048(t), 8 heads x 65] bf16 augmented with a ones column
     per head (softmax denominator via the same AV matmul).
  3. Per (head-pair, q-tile of 512): scoresT [k,q] in PSUM (2 heads packed
     into partition halves of the PE), exp on ScalarE (scale=1/8, fp32 in ->
     bf16 out, live query sub-ranges only), causal 0/1-mask multiply on the
     four diagonal 128x128 blocks only (VectorE). AV runs in the flipped
     orientation out[q(128 part), 65(free)] = P[k,qsub]^T @ Vh[k,65], one
     PSUM-accumulated chain per (q-subtile, head) packed 4-per-bank in a
     single 2-bank accumulator tile; dead (k>q) subtiles are skipped
     entirely. This costs 65 PE rows per (ktile, head, qsub) instead of the
     512 of the [65 part, q free] orientation. Normalization is a batched
     VectorE reciprocal of the 8 denominator columns plus two stride-0
     broadcast multiplies into attn [t, dloc] bf16.
  4. attn chunks are DMA-xbar-transposed into attnT [dloc, t]; out_proj
     matmuls (attnT-stationary vs W_out rows) write PSUM which is DMA'd
     straight to DRAM. Emitted one q-tile behind the last head-pair.

Projections for head-pair j+1 are emitted between attention units so the
TensorE fills exp-latency stalls with projection matmuls.
"""

import numpy as np

import concourse.bass as bass
import concourse.mybir as mybir
import concourse.tile as tile
from concourse.bass_utils import run_bass_kernel_spmd

DEBUG_DUMP = False

F32 = mybir.dt.float32
BF16 = mybir.dt.bfloat16
AX = mybir.AluOpType

T = 2048
D = 1024
HLOC = 8          # heads per core
DKH = 64
QT = 512          # query tile
NQT = T // QT     # 4
NDIN = D // 128   # 8
NMT = 4           # q/k feature m-tiles (512 local feats / 128)
VA = 65           # V cols per head incl. ones column
NTT = T // 128    # 16
EXP = mybir.ActivationFunctionType.Exp


_NOP_ID = [0]


def _split_multiwaits(nc, limit=1):
    """This toolchain's walrus rejects more than one sync-wait on an
    instruction ("Too many sync wait commands"), and rejects any sync-wait
    on the DMA-xpose descriptor. Move excess waits onto same-engine NOPs
    inserted immediately before the instruction — the engine sequencer
    executes them in program order, so semantics are preserved."""
    for f in nc.m.functions:
        for blk in f.blocks:
            new = []
            changed = False
            for inst in blk.instructions:
                si = inst.sync_info
                lim = 0 if type(inst).__name__ == "InstDmaTransposeAnt" else limit
                if si is not None and len(si.on_wait) > lim:
                    waits = list(si.on_wait)
                    inst.sync_info = mybir.SyncInfo(
                        on_wait=waits[:lim], on_update=list(si.on_update))
                    for w in waits[lim:]:
                        _NOP_ID[0] += 1
                        nop = mybir.InstNoOp(
                            name=f"waitnop-{_NOP_ID[0]}", ins=[], outs=[])
                        nop.engine = inst.engine
                        nop.sync_info = mybir.SyncInfo(on_wait=[w], on_update=[])
                        new.append(nop)
                    changed = True
                new.append(inst)
            if changed:
                blk.instructions = new


def build_nc():
    nc = bass.Bass()
    x_ext = nc.declare_dram_parameter("x", [T, D], F32, isOutput=False)
    w_ext = nc.declare_dram_parameter("W_qkv", [D, 3 * 512], F32, isOutput=False)
    b_ext = nc.declare_dram_parameter("b_qkv", [3 * 512], F32, isOutput=False)
    wo_ext = nc.declare_dram_parameter("W_out", [512, D], F32, isOutput=False)
    out_ext = nc.declare_dram_parameter("out", [T, D], F32, isOutput=True)
    if DEBUG_DUMP:
        dbg = {
            "xT": nc.declare_dram_parameter("d_xT", [128, NDIN * T], F32, isOutput=True),
            "qT": nc.declare_dram_parameter("d_qT", [128, NMT * T], F32, isOutput=True),
            "kT": nc.declare_dram_parameter("d_kT", [128, NMT * T], F32, isOutput=True),
            "vaug": nc.declare_dram_parameter("d_vaug", [128, NTT * HLOC * VA], F32, isOutput=True),
            "attn": nc.declare_dram_parameter("d_attn", [128, NTT * 512], F32, isOutput=True),
            "attnT": nc.declare_dram_parameter("d_attnT", [128, 4 * T], F32, isOutput=True),
        }

    with tile.TileContext(nc) as tc:
        with (
            tc.tile_pool(name="const", bufs=1) as constp,
            tc.tile_pool(name="big", bufs=1) as bigp,
        ):
            # causal 0/1 mask [128, 2x512] bf16 (head-duplicated triangle):
            # mask[p, (h, f)] = 1 if f >= p else 0. Diagonal 128x128 blocks
            # use the [:, :, 0:128] slice. Generated AFTER the first load
            # DMAs are issued (these ops run on Pool and would delay the
            # SWDGE descriptor generation of the x/W casting loads).
            maskt = constp.tile([128, 1024], BF16, tag="maskt")

            def emit_mask_gen():
                nc.gpsimd.memset(maskt, 1.0)
                mk3 = maskt.rearrange("p (h f) -> p h f", f=512)
                nc.gpsimd.affine_select(
                    out=mk3, in_=mk3,
                    compare_op=AX.is_ge, fill=0.0,
                    base=0, channel_multiplier=-1,
                    pattern=[[0, 2], [1, 512]],
                )

            # biases: per-partition vectors for q/k feature tiles, broadcast
            # tile for V (bias along the free dv axis); DMAs issued inside
            # the load phase (after the first x tiles) to keep the DMA
            # device free for the critical-path loads.
            bq_sb = constp.tile([128, NMT], F32, tag="bq")
            bk_sb = constp.tile([128, NMT], F32, tag="bk")
            bv_sb = constp.tile([128, 512], F32, tag="bv")

            def emit_bias_loads():
                nc.sync.dma_start(
                    out=bq_sb,
                    in_=b_ext[0:512].rearrange("(m p) -> p m", p=128))
                nc.sync.dma_start(
                    out=bk_sb,
                    in_=b_ext[512:1024].rearrange("(m p) -> p m", p=128))
                bv_src = b_ext[1024:1536]
                nc.sync.dma_start(
                    out=bv_sb,
                    in_=bass.AP(tensor=bv_src.tensor, offset=bv_src.offset,
                                ap=[[0, 128]] + list(bv_src.ap)),
                )

            # persistent activations / weights (all bf16)
            qT = bigp.tile([128, NMT * T], BF16, tag="qT")
            kT = bigp.tile([128, NMT * T], BF16, tag="kT")
            vaug = bigp.tile([128, NTT * HLOC * VA], BF16, tag="vaug")
            attn = bigp.tile([128, NTT * 512], BF16, tag="attn")
            attnT = bigp.tile([128, 4 * T], BF16, tag="attnT")
            woutb = bigp.tile([128, 4 * D], BF16, tag="woutb")
            wbf = bigp.tile([128, NDIN * 1536], BF16, tag="wbf")
            xT = bigp.tile([128, NDIN * T], BF16, tag="xT")

            with (
                tc.tile_pool(name="pjpsum", bufs=2, space="PSUM") as pjpsum,
            ):
                # ---- loads: gpsimd casting DMAs (f32->bf16 in flight,
                # batched 4 row-chunks per DMA to amortize SWDGE gen) +
                # ACT-issued DMA xbar-transposes for xT
                def cast_load(dst, dst_off, src, row0, nrow, width,
                              dst_stride=None, src_roww=None, src_col0=0):
                    # f32->bf16 casting DMA of `nrow` 128-row chunks; the
                    # source may be a column slice [src_col0, src_col0+width)
                    # of rows with full width src_roww.
                    sw = src_roww or width
                    ds = dst_stride or width
                    nc.gpsimd.dma_start(
                        out=bass.AP(
                            tensor=dst.tensor, offset=dst.offset + dst_off,
                            ap=[list(dst.ap[0]), [ds, nrow], [1, width]]),
                        in_=bass.AP(
                            tensor=src.tensor,
                            offset=src.offset + row0 * sw + src_col0,
                            ap=[[sw, 128], [128 * sw, nrow], [1, width]]),
                    )

                # 2 row-chunks (256 descriptors) per DMA so 4 fit in the
                # SWDGE descriptor ring; x group 0 first (the xT transposes
                # are on the critical path), weights interleaved behind it.
                # x: Pool casting DMAs (f32->bf16 in flight, 4 t-tiles per
                # DMA) -> bf16 PE transposes (1 cycle/row) into a bf16 PSUM
                # bank -> one merged DVE copy per t-tile (2-byte 2x mode).
                # W: Pool casting DMAs, V columns first (vproj needs only
                # those), interleaved behind x group 0.
                from concourse.masks import make_identity
                ident = constp.tile([128, 128], BF16, tag="ident")
                with (
                    tc.tile_pool(name="xstage", bufs=4) as xstage,
                    tc.tile_pool(name="tpsum", bufs=2, space="PSUM") as tpsum,
                ):
                    def load_xg(g):
                        xg = xstage.tile([128, 4 * D], BF16, tag="xg")
                        cast_load(xg, 0, x_ext[0:1, :], 512 * g, 4, D)
                        return xg

                    def xpose_x(xg, r):
                        tp = tpsum.tile([128, 1024], BF16, tag="tp")
                        for dj in range(NDIN):
                            nc.tensor.transpose(
                                tp[:, dj * 128:(dj + 1) * 128],
                                xg[:, r * D + dj * 128:r * D + (dj + 1) * 128],
                                ident)
                        return tp

                    def xpose_tts(xg, g):
                        for r in range(4):
                            tp = xpose_x(xg, r)
                            nc.vector.tensor_copy(
                                bass.AP(tensor=xT.tensor,
                                        offset=xT.offset + (4 * g + r) * 128,
                                        ap=[list(xT.ap[0]), [T, NDIN],
                                            [1, 128]]),
                                tp.rearrange("p (dj t) -> p dj t", t=128))

                    xg0 = load_xg(0)
                    cast_load(wbf, 1024, w_ext[0:1, :], 0, 4, 512,
                              dst_stride=1536, src_roww=1536, src_col0=1024)
                    cast_load(wbf, 4 * 1536 + 1024, w_ext[0:1, :], 512, 4, 512,
                              dst_stride=1536, src_roww=1536, src_col0=1024)
                    make_identity(nc, ident)
                    emit_mask_gen()
                    xpose_tts(xg0, 0)
                    cast_load(wbf, 0, w_ext[0:1, :], 0, 4, 1024,
                              dst_stride=1536, src_roww=1536, src_col0=0)
                    cast_load(wbf, 4 * 1536, w_ext[0:1, :], 512, 4, 1024,
                              dst_stride=1536, src_roww=1536, src_col0=0)
                    emit_bias_loads()
                    xg1 = load_xg(1)
                    xpose_tts(xg1, 1)
                    xg2 = load_xg(2)
                    xpose_tts(xg2, 2)
                    cast_load(woutb, 0, wo_ext[0:1, :], 0, 4, D)
                    xg3 = load_xg(3)
                    xpose_tts(xg3, 3)

                def qk_chain(mt, sec, n):
                    dst, bias = (qT, bq_sb) if sec == 0 else (kT, bk_sb)
                    ps = pjpsum.tile([128, 512], F32, tag="pj")
                    for kk in range(NDIN):
                        nc.tensor.matmul(
                            ps,
                            lhsT=wbf[:, kk * 1536 + sec * 512 + mt * 128:
                                     kk * 1536 + sec * 512 + (mt + 1) * 128],
                            rhs=xT[:, kk * T + n * 512: kk * T + (n + 1) * 512],
                            start=(kk == 0), stop=(kk == NDIN - 1),
                        )
                    nc.vector.tensor_scalar(
                        out=dst[:, mt * T + n * 512: mt * T + (n + 1) * 512],
                        in0=ps, scalar1=bias[:, mt:mt + 1], scalar2=None,
                        op0=AX.add)

                def emit_qkproj(mt, ns=None):
                    for sec in (0, 1):
                        for n in (range(NQT) if ns is None else ns):
                            qk_chain(mt, sec, n)

                def emit_vproj(tts):
                    for tt in tts:
                        ps = pjpsum.tile([128, 512], F32, tag="pj")
                        for kk in range(NDIN):
                            nc.tensor.matmul(
                                ps,
                                lhsT=xT[:, kk * T + tt * 128: kk * T + (tt + 1) * 128],
                                rhs=wbf[:, kk * 1536 + 1024: kk * 1536 + 1536],
                                start=(kk == 0), stop=(kk == NDIN - 1),
                            )
                        blk = vaug[:, tt * (HLOC * VA):(tt + 1) * (HLOC * VA)]
                        blk3 = blk.rearrange("p (h c) -> p h c", c=VA)
                        nc.vector.tensor_tensor(
                            out=blk3[:, :, 0:64],
                            in0=ps.rearrange("p (h c) -> p h c", c=64),
                            in1=bv_sb.rearrange("p (h c) -> p h c", c=64),
                            op=AX.add)
                        nc.vector.memset(blk3[:, :, 64:65], 1.0)

                with (
                    tc.tile_pool(name="scps", bufs=2, space="PSUM") as scps,
                    tc.tile_pool(name="avps", bufs=1, space="PSUM") as avps,
                    tc.tile_pool(name="ptp", bufs=10) as ptp,
                    tc.tile_pool(name="recp", bufs=2) as recp,
                    tc.tile_pool(name="yo", bufs=3) as yo,
                ):
                    m3 = maskt.rearrange("p (h q) -> p h q", q=512)

                    def av_off(j, par):
                        s = 2 * j + par
                        return (s // 4) * 512 + (s % 4) * 65

                    def emit_unit(hp, qt, fillers=None, per_slot=1,
                                  tail=False):
                        def fill(k):
                            for _ in range(k):
                                if fillers:
                                    fillers.popleft()()

                        def av_mm(kt, j, par, pt_t):
                            h = 2 * hp + par
                            off = av_off(j, par)
                            # PSUM start/stop are per 2KB bank (zero
                            # region): exactly one start (zeroes the bank)
                            # and one stop per bank; sibling chains ride.
                            nc.tensor.matmul(
                                avt[:, off:off + VA],
                                lhsT=pt_t[:, par * 512 + j * 128:
                                          par * 512 + (j + 1) * 128],
                                rhs=vaug[:, kt * (HLOC * VA) + h * VA:
                                         kt * (HLOC * VA) + (h + 1) * VA],
                                start=(kt == 0 and par == 0 and j % 2 == 0),
                                stop=(par == 1 and j % 2 == 1
                                      and kt == 4 * qt + j),
                            )

                        nkt = 4 * (qt + 1)
                        avt = avps.tile([128, 1024], F32, tag="av")
                        for c0 in range(0, nkt, 8):
                            if hp == 3:
                                fill(1)
                            chunk = list(range(c0, min(c0 + 8, nkt)))
                            pts = {}
                            for kt in chunk:
                                # diagonal k-tile (i >= 0): only queries
                                # f >= 128*i are live
                                i = kt - (nkt - 4)
                                lo_q = max(0, 128 * i)
                                ps = scps.tile([128, 1024], F32, tag="sc")
                                for par in range(2):
                                    lo, hi = par * 64, par * 64 + 64
                                    nc.tensor.matmul(
                                        ps[:, par * 512 + lo_q:(par + 1) * 512],
                                        lhsT=kT[lo:hi, hp * T + kt * 128:
                                                hp * T + (kt + 1) * 128],
                                        rhs=qT[lo:hi, hp * T + qt * 512 + lo_q:
                                               hp * T + (qt + 1) * 512],
                                        start=True, stop=True,
                                    )
                                pt_t = ptp.tile([128, 1024], BF16, tag="pt")
                                ps3 = ps.rearrange("p (h q) -> p h q", q=512)
                                pt3 = pt_t.rearrange("p (h q) -> p h q", q=512)
                                nc.scalar.activation(
                                    pt3[:, :, lo_q:512], ps3[:, :, lo_q:512],
                                    EXP, bias=0.0, scale=0.125)
                                if i >= 0:
                                    # mask the diagonal 128x128 block (both
                                    # heads): q-subtile j == i
                                    blk = bass.AP(
                                        tensor=pt_t.tensor,
                                        offset=pt_t.offset + lo_q,
                                        ap=[list(pt_t.ap[0]), [512, 2], [1, 128]])
                                    mblk = bass.AP(
                                        tensor=maskt.tensor,
                                        offset=maskt.offset,
                                        ap=[list(maskt.ap[0]), [512, 2], [1, 128]])
                                    nc.vector.tensor_tensor(
                                        out=blk, in0=blk, in1=mblk, op=AX.mult)
                                pts[kt] = pt_t
                            fill(per_slot)
                            last = tail and c0 + 8 >= nkt
                            if not last:
                                for kt in chunk:
                                    for j in range(4):
                                        if kt > 4 * qt + j:
                                            continue  # fully-dead block
                                        for par in range(2):
                                            av_mm(kt, j, par, pts[kt])
                            else:
                                # tail: finish bank 0 (q-subtiles 0,1) first
                                # so its normalize / attnT transpose /
                                # out-proj overlap bank 1's AV matmuls
                                fill(1)
                                for bank in range(2):
                                    if bank == 1:
                                        fill(2)
                                    for kt in chunk:
                                        for j in (2 * bank, 2 * bank + 1):
                                            if kt > 4 * qt + j:
                                                continue
                                            for par in range(2):
                                                av_mm(kt, j, par, pts[kt])
                                    normalize(hp, qt, avt, bank=bank)
                                    xpose_tt(4 * qt + 2 * bank)
                                    xpose_tt(4 * qt + 2 * bank + 1)
                        if not tail:
                            normalize(hp, qt, avt)

                    def normalize(hp, qt, avt, bank=None):
                        # rc[p, s] = 1 / denom(slot s); attn chunk cols
                        # (hp, par) of t-chunks 4qt..4qt+3 (or one bank's 2)
                        if bank is None:
                            rc = recp.tile([128, 8], F32, tag="rc")
                            nc.vector.reciprocal(
                                rc,
                                bass.AP(tensor=avt.tensor,
                                        offset=avt.offset + 64,
                                        ap=[list(avt.ap[0]), [512, 2],
                                            [130, 2], [65, 2]]))
                            jd, base = [512, 2], 0
                            rjd = [4, 2]
                        else:
                            rc = recp.tile([128, 4], F32, tag="rc")
                            nc.vector.reciprocal(
                                rc,
                                bass.AP(tensor=avt.tensor,
                                        offset=avt.offset + bank * 512 + 64,
                                        ap=[list(avt.ap[0]), [130, 2],
                                            [65, 2]]))
                            jd, base = [130, 2], bank * 512
                            rjd = [2, 2]
                        nj = 4 if bank is None else 2
                        j0 = 0 if bank is None else 2 * bank
                        for par in range(2):
                            if bank is None:
                                dstd = [[1024, 2], [512, 2]]
                                srcd = [[512, 2], [130, 2]]
                                rcd = [[4, 2], [2, 2]]
                            else:
                                dstd = [[512, 2]]
                                srcd = [[130, 2]]
                                rcd = [[2, 2]]
                            dst = bass.AP(
                                tensor=attn.tensor,
                                offset=attn.offset + (4 * qt + j0) * 512
                                + (hp * 2 + par) * 64,
                                ap=[list(attn.ap[0])] + dstd + [[1, 64]])
                            src = bass.AP(
                                tensor=avt.tensor,
                                offset=avt.offset + base + par * 65,
                                ap=[list(avt.ap[0])] + srcd + [[1, 64]])
                            rcb = bass.AP(
                                tensor=rc.tensor, offset=rc.offset + par,
                                ap=[list(rc.ap[0])] + rcd + [[0, 64]])
                            nc.vector.tensor_tensor(
                                out=dst, in0=src, in1=rcb, op=AX.mult)

                    def xpose_tt(tt):
                        nc.sync.dma_start_transpose(
                            out=bass.AP(
                                tensor=attnT.tensor,
                                offset=attnT.offset + tt * 128,
                                ap=[list(attnT.ap[0]), [T, 4], [1, 128]]),
                            in_=attn[:, tt * 512:(tt + 1) * 512])

                    def emit_attnT(qt):
                        for j in range(4):
                            xpose_tt(4 * qt + j)

                    def out_chain(mt, n):
                        ps = pjpsum.tile([128, 512], F32, tag="pj", name="y")
                        for kk in range(4):
                            nc.tensor.matmul(
                                ps,
                                lhsT=attnT[:, kk * T + mt * 128:
                                           kk * T + (mt + 1) * 128],
                                rhs=woutb[:, kk * D + n * 512:
                                          kk * D + (n + 1) * 512],
                                start=(kk == 0), stop=(kk == 3))
                        yt = yo.tile([128, 512], F32, tag="yt", name="yt")
                        nc.vector.tensor_copy(yt, ps)
                        nc.sync.dma_start(
                            out=out_ext[mt * 128:(mt + 1) * 128,
                                        n * 512:(n + 1) * 512], in_=yt)

                    def emit_outproj(mts):
                        for mt in mts:
                            for n in range(2):
                                out_chain(mt, n)

                    from collections import deque

                    # Projection chains for head-pair hp+1 (and, on the last
                    # row, out-proj chains) are drained as fillers between an
                    # attention chunk's exp and its AV matmuls, so the PE has
                    # exp-independent work exactly where it would stall.
                    for hp in range(HLOC // 2):
                        fillers = deque()
                        if hp < 3:
                            for sec in (0, 1):
                                for n in range(NQT):
                                    fillers.append(
                                        (lambda m, s, nn:
                                         lambda: qk_chain(m, s, nn))(
                                             hp + 1, sec, n))
                        for qt in range(NQT):
                            if hp == 0:
                                if qt == 0:
                                    emit_vproj(range(0, 4))
                                    emit_qkproj(0, ns=[0])
                                else:
                                    emit_qkproj(0, ns=[qt])
                                    emit_vproj(range(4 * qt, 4 * qt + 4))
                            if hp == 3 and qt >= 1:
                                for mt in range(4 * (qt - 1), 4 * qt):
                                    for n in range(2):
                                        fillers.append(
                                            (lambda m, nn:
                                             lambda: out_chain(m, nn))(mt, n))
                            tail = hp == 3 and qt == 3
                            emit_unit(hp, qt, fillers,
                                      per_slot=(3 if hp == 3 else
                                                2 if qt >= 2 else 0),
                                      tail=tail)
                            if hp == 3 and not tail:
                                emit_attnT(qt)
                        while fillers:
                            fillers.popleft()()
                    emit_outproj(range(12, 16))
                    if DEBUG_DUMP:
                        with tc.tile_pool(name="dbgp", bufs=2) as dbgp:
                            for nm, src in (("xT", xT), ("qT", qT), ("kT", kT),
                                            ("vaug", vaug), ("attn", attn),
                                            ("attnT", attnT)):
                                w = src.shape[1]
                                for c0 in range(0, w, 2048):
                                    cw = min(2048, w - c0)
                                    t = dbgp.tile([128, cw], F32, tag="dbg")
                                    nc.vector.tensor_copy(t, src[:, c0:c0 + cw])
                                    nc.sync.dma_start(
                                        out=dbg[nm][0:128, c0:c0 + cw], in_=t)

    _split_multiwaits(nc)
    return nc


_NC_CACHE = {}


def get_nc():
    if "nc" not in _NC_CACHE:
        _NC_CACHE["nc"] = build_nc()
    return _NC_CACHE["nc"]


def make_in_maps(x, W_qkv, b_qkv, W_out):
    in_maps = []
    for c in range(8):
        b, g = c // 2, c % 2
        s = slice(512 * g, 512 * (g + 1))
        wslice = np.concatenate(
            [W_qkv[:, 512 * g:512 * (g + 1)],
             W_qkv[:, 1024 + 512 * g:1024 + 512 * (g + 1)],
             W_qkv[:, 2048 + 512 * g:2048 + 512 * (g + 1)]], axis=1)
        bslice = np.concatenate(
            [b_qkv[512 * g:512 * (g + 1)],
             b_qkv[1024 + 512 * g:1024 + 512 * (g + 1)],
             b_qkv[2048 + 512 * g:2048 + 512 * (g + 1)]])
        in_maps.append({
            "x": np.ascontiguousarray(x[b], dtype=np.float32),
            "W_qkv": np.ascontiguousarray(wslice, dtype=np.float32),
            "b_qkv": np.ascontiguousarray(bslice, dtype=np.float32),
            "W_out": np.ascontiguousarray(W_out[s], dtype=np.float32),
        })
    return in_maps


def kernel(x, W_qkv, b_qkv, W_out, b_out):
    x = np.asarray(x)
    W_qkv = np.asarray(W_qkv)
    b_qkv = np.asarray(b_qkv)
    W_out = np.asarray(W_out)
    b_out = np.asarray(b_out)
    nc = get_nc()
    in_maps = make_in_maps(x, W_qkv, b_qkv, W_out)
    res = run_bass_kernel_spmd(nc, in_maps, core_ids=list(range(8))).results
    out = np.stack(
        [res[2 * b]["out"] + res[2 * b + 1]["out"] for b in range(4)], axis=0)
    out = out + b_out[None, None, :]
    return out.astype(np.float32)


# revision 49
# speedup vs baseline: 1.2595x; 1.0680x over previous
"""Causal self-attention (B=4, T=2048, D=1024, H=16) on 8 TRN2 NeuronCores.

Sharding: core c -> (batch b = c//2, head-group g = c%2 of 8 heads).
Each core computes QKV projection for its 8 heads, causal attention, and a
partial out-projection (its heads' rows of W_out). The two partials per batch
are summed on the host during unshard (the "all-reduce after out_proj" of the
tensor-parallel scheme).

Per-core program (identical SPMD on all 8 cores):
  1. Loads: gpsimd (SWDGE) casting DMAs bring x, W_qkv, W_out from DRAM f32
     into SBUF as bf16 directly; xT [1024(d), 2048(t)] is produced by DMA
     xbar-transposes (16x128 tiles) issued on the ACT queue. No PE transposes
     and no engine-side casts anywhere.
  2. qT/kT = W^T-stationary matmuls -> [512(feat), 2048(t)] bf16 with the
     bias added during the PSUM->SBUF move on VectorE; V = xT-stationary
     matmuls -> [2048(t), 8 heads x 65] bf16 augmented with a ones column
     per head (softmax denominator via the same AV matmul).
  3. Per (head-pair, q-tile of 512): scoresT [k,q] in PSUM (2 heads packed
     into partition halves of the PE), exp on ScalarE (scale=1/8, fp32 in ->
     bf16 out, live query sub-ranges only), causal 0/1-mask multiply on the
     four diagonal 128x128 blocks only (VectorE). AV runs in the flipped
     orientation out[q(128 part), 65(free)] = P[k,qsub]^T @ Vh[k,65], one
     PSUM-accumulated chain per (q-subtile, head) packed 4-per-bank in a
     single 2-bank accumulator tile; dead (k>q) subtiles are skipped
     entirely. This costs 65 PE rows per (ktile, head, qsub) instead of the
     512 of the [65 part, q free] orientation. Normalization is a batched
     VectorE reciprocal of the 8 denominator columns plus two stride-0
     broadcast multiplies into attn [t, dloc] bf16.
  4. attn chunks are DMA-xbar-transposed into attnT [dloc, t]; out_proj
     matmuls (attnT-stationary vs W_out rows) write PSUM which is DMA'd
     straight to DRAM. Emitted one q-tile behind the last head-pair.

Projections for head-pair j+1 are emitted between attention units so the
TensorE fills exp-latency stalls with projection matmuls.
"""

import numpy as np

import concourse.bass as bass
import concourse.mybir as mybir
import concourse.tile as tile
from concourse.bass_utils import run_bass_kernel_spmd

DEBUG_DUMP = False

F32 = mybir.dt.float32
BF16 = mybir.dt.bfloat16
AX = mybir.AluOpType

T = 2048
D = 1024
HLOC = 8          # heads per core
DKH = 64
QT = 512          # query tile
NQT = T // QT     # 4
NDIN = D // 128   # 8
NMT = 4           # q/k feature m-tiles (512 local feats / 128)
VA = 65           # V cols per head incl. ones column
NTT = T // 128    # 16
EXP = mybir.ActivationFunctionType.Exp


_NOP_ID = [0]


def _split_multiwaits(nc, limit=1):
    """This toolchain's walrus rejects more than one sync-wait on an
    instruction ("Too many sync wait commands"), and rejects any sync-wait
    on the DMA-xpose descriptor. Move excess waits onto same-engine NOPs
    inserted immediately before the instruction — the engine sequencer
    executes them in program order, so semantics are preserved."""
    for f in nc.m.functions:
        for blk in f.blocks:
            new = []
            changed = False
            for inst in blk.instructions:
                si = inst.sync_info
                lim = 0 if type(inst).__name__ == "InstDmaTransposeAnt" else limit
                if si is not None and len(si.on_wait) > lim:
                    waits = list(si.on_wait)
                    inst.sync_info = mybir.SyncInfo(
                        on_wait=waits[:lim], on_update=list(si.on_update))
                    for w in waits[lim:]:
                        _NOP_ID[0] += 1
                        nop = mybir.InstNoOp(
                            name=f"waitnop-{_NOP_ID[0]}", ins=[], outs=[])
                        nop.engine = inst.engine
                        nop.sync_info = mybir.SyncInfo(on_wait=[w], on_update=[])
                        new.append(nop)
                    changed = True
                new.append(inst)
            if changed:
                blk.instructions = new


def build_nc():
    nc = bass.Bass()
    x_ext = nc.declare_dram_parameter("x", [T, D], F32, isOutput=False)
    w_ext = nc.declare_dram_parameter("W_qkv", [D, 3 * 512], F32, isOutput=False)
    b_ext = nc.declare_dram_parameter("b_qkv", [3 * 512], F32, isOutput=False)
    wo_ext = nc.declare_dram_parameter("W_out", [512, D], F32, isOutput=False)
    out_ext = nc.declare_dram_parameter("out", [T, D], F32, isOutput=True)
    if DEBUG_DUMP:
        dbg = {
            "xT": nc.declare_dram_parameter("d_xT", [128, NDIN * T], F32, isOutput=True),
            "qT": nc.declare_dram_parameter("d_qT", [128, NMT * T], F32, isOutput=True),
            "kT": nc.declare_dram_parameter("d_kT", [128, NMT * T], F32, isOutput=True),
            "vaug": nc.declare_dram_parameter("d_vaug", [128, NTT * HLOC * VA], F32, isOutput=True),
            "attn": nc.declare_dram_parameter("d_attn", [128, NTT * 512], F32, isOutput=True),
            "attnT": nc.declare_dram_parameter("d_attnT", [128, 4 * T], F32, isOutput=True),
        }

    with tile.TileContext(nc) as tc:
        with (
            tc.tile_pool(name="const", bufs=1) as constp,
            tc.tile_pool(name="big", bufs=1) as bigp,
        ):
            # causal 0/1 mask [128, 2x512] bf16 (head-duplicated triangle):
            # mask[p, (h, f)] = 1 if f >= p else 0. Diagonal 128x128 blocks
            # use the [:, :, 0:128] slice. Generated AFTER the first load
            # DMAs are issued (these ops run on Pool and would delay the
            # SWDGE descriptor generation of the x/W casting loads).
            maskt = constp.tile([128, 1024], BF16, tag="maskt")

            def emit_mask_gen():
                nc.gpsimd.memset(maskt, 1.0)
                mk3 = maskt.rearrange("p (h f) -> p h f", f=512)
                nc.gpsimd.affine_select(
                    out=mk3, in_=mk3,
                    compare_op=AX.is_ge, fill=0.0,
                    base=0, channel_multiplier=-1,
                    pattern=[[0, 2], [1, 512]],
                )

            # biases: per-partition vectors for q/k feature tiles, broadcast
            # tile for V (bias along the free dv axis); DMAs issued inside
            # the load phase (after the first x tiles) to keep the DMA
            # device free for the critical-path loads.
            bq_sb = constp.tile([128, NMT], F32, tag="bq")
            bk_sb = constp.tile([128, NMT], F32, tag="bk")
            bv_sb = constp.tile([128, 512], F32, tag="bv")

            def emit_bias_loads():
                nc.sync.dma_start(
                    out=bq_sb,
                    in_=b_ext[0:512].rearrange("(m p) -> p m", p=128))
                nc.sync.dma_start(
                    out=bk_sb,
                    in_=b_ext[512:1024].rearrange("(m p) -> p m", p=128))
                bv_src = b_ext[1024:1536]
                nc.sync.dma_start(
                    out=bv_sb,
                    in_=bass.AP(tensor=bv_src.tensor, offset=bv_src.offset,
                                ap=[[0, 128]] + list(bv_src.ap)),
                )

            # persistent activations / weights (all bf16)
            qT = bigp.tile([128, NMT * T], BF16, tag="qT")
            kT = bigp.tile([128, NMT * T], BF16, tag="kT")
            vaug = bigp.tile([128, NTT * HLOC * VA], BF16, tag="vaug")
            attn = bigp.tile([128, NTT * 512], BF16, tag="attn")
            attnT = bigp.tile([128, 4 * T], BF16, tag="attnT")
            woutb = bigp.tile([128, 4 * D], BF16, tag="woutb")
            wbf = bigp.tile([128, NDIN * 1536], BF16, tag="wbf")
            xT = bigp.tile([128, NDIN * T], BF16, tag="xT")

            with (
                tc.tile_pool(name="pjpsum", bufs=2, space="PSUM") as pjpsum,
            ):
                # ---- loads: gpsimd casting DMAs (f32->bf16 in flight,
                # batched 4 row-chunks per DMA to amortize SWDGE gen) +
                # ACT-issued DMA xbar-transposes for xT
                def cast_load(dst, dst_off, src, row0, nrow, width,
                              dst_stride=None, src_roww=None, src_col0=0):
                    # f32->bf16 casting DMA of `nrow` 128-row chunks; the
                    # source may be a column slice [src_col0, src_col0+width)
                    # of rows with full width src_roww.
                    sw = src_roww or width
                    ds = dst_stride or width
                    nc.gpsimd.dma_start(
                        out=bass.AP(
                            tensor=dst.tensor, offset=dst.offset + dst_off,
                            ap=[list(dst.ap[0]), [ds, nrow], [1, width]]),
                        in_=bass.AP(
                            tensor=src.tensor,
                            offset=src.offset + row0 * sw + src_col0,
                            ap=[[sw, 128], [128 * sw, nrow], [1, width]]),
                    )

                # 2 row-chunks (256 descriptors) per DMA so 4 fit in the
                # SWDGE descriptor ring; x group 0 first (the xT transposes
                # are on the critical path), weights interleaved behind it.
                # x: Pool casting DMAs (f32->bf16 in flight, 4 t-tiles per
                # DMA) -> bf16 PE transposes (1 cycle/row) into a bf16 PSUM
                # bank -> one merged DVE copy per t-tile (2-byte 2x mode).
                # W: Pool casting DMAs, V columns first (vproj needs only
                # those), interleaved behind x group 0.
                from concourse.masks import make_identity
                ident = constp.tile([128, 128], BF16, tag="ident")
                with (
                    tc.tile_pool(name="xstage", bufs=4) as xstage,
                    tc.tile_pool(name="tpsum", bufs=2, space="PSUM") as tpsum,
                ):
                    def load_xg(g, nrow=4, row0=None):
                        xg = xstage.tile([128, nrow * D], BF16, tag="xg")
                        cast_load(xg, 0, x_ext[0:1, :],
                                  512 * g if row0 is None else row0, nrow, D)
                        return xg

                    def xpose_x(xg, r):
                        tp = tpsum.tile([128, 1024], BF16, tag="tp")
                        for dj in range(NDIN):
                            nc.tensor.transpose(
                                tp[:, dj * 128:(dj + 1) * 128],
                                xg[:, r * D + dj * 128:r * D + (dj + 1) * 128],
                                ident)
                        return tp

                    def xpose_tts(xg, g):
                        for r in range(4):
                            tp = xpose_x(xg, r)
                            nc.vector.tensor_copy(
                                bass.AP(tensor=xT.tensor,
                                        offset=xT.offset + (4 * g + r) * 128,
                                        ap=[list(xT.ap[0]), [T, NDIN],
                                            [1, 128]]),
                                tp.rearrange("p (dj t) -> p dj t", t=128))

                    xg0a = load_xg(0, nrow=1, row0=0)
                    xg0b = load_xg(0, nrow=3, row0=128)
                    cast_load(wbf, 1024, w_ext[0:1, :], 0, 4, 512,
                              dst_stride=1536, src_roww=1536, src_col0=1024)
                    cast_load(wbf, 4 * 1536 + 1024, w_ext[0:1, :], 512, 4, 512,
                              dst_stride=1536, src_roww=1536, src_col0=1024)
                    make_identity(nc, ident)
                    emit_mask_gen()
                    tp = xpose_x(xg0a, 0)
                    nc.vector.tensor_copy(
                        bass.AP(tensor=xT.tensor, offset=xT.offset,
                                ap=[list(xT.ap[0]), [T, NDIN], [1, 128]]),
                        tp.rearrange("p (dj t) -> p dj t", t=128))
                    for r in range(3):
                        tp = xpose_x(xg0b, r)
                        nc.vector.tensor_copy(
                            bass.AP(tensor=xT.tensor,
                                    offset=xT.offset + (1 + r) * 128,
                                    ap=[list(xT.ap[0]), [T, NDIN], [1, 128]]),
                            tp.rearrange("p (dj t) -> p dj t", t=128))
                    cast_load(wbf, 0, w_ext[0:1, :], 0, 4, 1024,
                              dst_stride=1536, src_roww=1536, src_col0=0)
                    cast_load(wbf, 4 * 1536, w_ext[0:1, :], 512, 4, 1024,
                              dst_stride=1536, src_roww=1536, src_col0=0)
                    emit_bias_loads()
                    xg1 = load_xg(1)
                    xpose_tts(xg1, 1)
                    xg2 = load_xg(2)
                    xpose_tts(xg2, 2)
                    cast_load(woutb, 0, wo_ext[0:1, :], 0, 4, D)
                    xg3 = load_xg(3)
                    xpose_tts(xg3, 3)

                def qk_chain(mt, sec, n):
                    dst, bias = (qT, bq_sb) if sec == 0 else (kT, bk_sb)
                    ps = pjpsum.tile([128, 512], F32, tag="pj")
                    for kk in range(NDIN):
                        nc.tensor.matmul(
                            ps,
                            lhsT=wbf[:, kk * 1536 + sec * 512 + mt * 128:
                                     kk * 1536 + sec * 512 + (mt + 1) * 128],
                            rhs=xT[:, kk * T + n * 512: kk * T + (n + 1) * 512],
                            start=(kk == 0), stop=(kk == NDIN - 1),
                        )
                    nc.vector.tensor_scalar(
                        out=dst[:, mt * T + n * 512: mt * T + (n + 1) * 512],
                        in0=ps, scalar1=bias[:, mt:mt + 1], scalar2=None,
                        op0=AX.add)

                def emit_qkproj(mt, ns=None):
                    for sec in (0, 1):
                        for n in (range(NQT) if ns is None else ns):
                            qk_chain(mt, sec, n)

                def emit_vproj(tts):
                    for tt in tts:
                        ps = pjpsum.tile([128, 512], F32, tag="pj")
                        for kk in range(NDIN):
                            nc.tensor.matmul(
                                ps,
                                lhsT=xT[:, kk * T + tt * 128: kk * T + (tt + 1) * 128],
                                rhs=wbf[:, kk * 1536 + 1024: kk * 1536 + 1536],
                                start=(kk == 0), stop=(kk == NDIN - 1),
                            )
                        blk = vaug[:, tt * (HLOC * VA):(tt + 1) * (HLOC * VA)]
                        blk3 = blk.rearrange("p (h c) -> p h c", c=VA)
                        nc.vector.tensor_tensor(
                            out=blk3[:, :, 0:64],
                            in0=ps.rearrange("p (h c) -> p h c", c=64),
                            in1=bv_sb.rearrange("p (h c) -> p h c", c=64),
                            op=AX.add)
                        nc.vector.memset(blk3[:, :, 64:65], 1.0)

                with (
                    tc.tile_pool(name="scps", bufs=2, space="PSUM") as scps,
                    tc.tile_pool(name="avps", bufs=1, space="PSUM") as avps,
                    tc.tile_pool(name="ptp", bufs=10) as ptp,
                    tc.tile_pool(name="recp", bufs=2) as recp,
                    tc.tile_pool(name="yo", bufs=8) as yo,
                ):
                    m3 = maskt.rearrange("p (h q) -> p h q", q=512)

                    def av_off(j, par):
                        s = 2 * j + par
                        return (s // 4) * 512 + (s % 4) * 65

                    def emit_unit(hp, qt, fillers=None, per_slot=1,
                                  tail=False):
                        def fill(k):
                            for _ in range(k):
                                if fillers:
                                    fillers.popleft()()

                        def av_mm(kt, j, par, pt_t):
                            h = 2 * hp + par
                            off = av_off(j, par)
                            # PSUM start/stop are per 2KB bank (zero
                            # region): exactly one start (zeroes the bank)
                            # and one stop per bank; sibling chains ride.
                            nc.tensor.matmul(
                                avt[:, off:off + VA],
                                lhsT=pt_t[:, par * 512 + j * 128:
                                          par * 512 + (j + 1) * 128],
                                rhs=vaug[:, kt * (HLOC * VA) + h * VA:
                                         kt * (HLOC * VA) + (h + 1) * VA],
                                start=(kt == 0 and par == 0 and j % 2 == 0),
                                stop=(par == 1 and j % 2 == 1
                                      and kt == 4 * qt + j),
                            )

                        nkt = 4 * (qt + 1)
                        avt = avps.tile([128, 1024], F32, tag="av")
                        for c0 in range(0, nkt, 8):
                            is_last = tail and c0 + 8 >= nkt
                            chunk = list(range(c0, min(c0 + 8, nkt)))
                            pts = {}
                            for kt in chunk:
                                # diagonal k-tile (i >= 0): only queries
                                # f >= 128*i are live
                                i = kt - (nkt - 4)
                                lo_q = max(0, 128 * i)
                                ps = scps.tile([128, 1024], F32, tag="sc")
                                for par in range(2):
                                    lo, hi = par * 64, par * 64 + 64
                                    nc.tensor.matmul(
                                        ps[:, par * 512 + lo_q:(par + 1) * 512],
                                        lhsT=kT[lo:hi, hp * T + kt * 128:
                                                hp * T + (kt + 1) * 128],
                                        rhs=qT[lo:hi, hp * T + qt * 512 + lo_q:
                                               hp * T + (qt + 1) * 512],
                                        start=True, stop=True,
                                    )
                                pt_t = ptp.tile([128, 1024], BF16, tag="pt")
                                ps3 = ps.rearrange("p (h q) -> p h q", q=512)
                                pt3 = pt_t.rearrange("p (h q) -> p h q", q=512)
                                nc.scalar.activation(
                                    pt3[:, :, lo_q:512], ps3[:, :, lo_q:512],
                                    EXP, bias=0.0, scale=0.125)
                                if i >= 0:
                                    # mask the diagonal 128x128 block (both
                                    # heads): q-subtile j == i
                                    blk = bass.AP(
                                        tensor=pt_t.tensor,
                                        offset=pt_t.offset + lo_q,
                                        ap=[list(pt_t.ap[0]), [512, 2], [1, 128]])
                                    mblk = bass.AP(
                                        tensor=maskt.tensor,
                                        offset=maskt.offset,
                                        ap=[list(maskt.ap[0]), [512, 2], [1, 128]])
                                    nc.vector.tensor_tensor(
                                        out=blk, in0=blk, in1=mblk, op=AX.mult)
                                pts[kt] = pt_t
                            fill(1 if is_last else per_slot)
                            if not is_last:
                                for kt in chunk:
                                    for j in range(4):
                                        if kt > 4 * qt + j:
                                            continue  # fully-dead block
                                        for par in range(2):
                                            av_mm(kt, j, par, pts[kt])
                            else:
                                # tail: finish bank 0 (q-subtiles 0,1) first
                                # so its normalize / attnT transpose /
                                # out-proj overlap bank 1's AV matmuls;
                                # leftover filler chains are drained right
                                # after each bank's transposes to cover the
                                # transpose latency before out-proj starts
                                for bank in range(2):
                                    for kt in chunk:
                                        for j in (2 * bank, 2 * bank + 1):
                                            if kt > 4 * qt + j:
                                                continue
                                            for par in range(2):
                                                av_mm(kt, j, par, pts[kt])
                                    normalize(hp, qt, avt, bank=bank)
                                    xpose_tt(4 * qt + 2 * bank)
                                    xpose_tt(4 * qt + 2 * bank + 1)
                                    fill(2)
                        if not tail:
                            normalize(hp, qt, avt)

                    def normalize(hp, qt, avt, bank=None):
                        # rc[p, s] = 1 / denom(slot s); attn chunk cols
                        # (hp, par) of t-chunks 4qt..4qt+3 (or one bank's 2)
                        if bank is None:
                            rc = recp.tile([128, 8], F32, tag="rc")
                            nc.vector.reciprocal(
                                rc,
                                bass.AP(tensor=avt.tensor,
                                        offset=avt.offset + 64,
                                        ap=[list(avt.ap[0]), [512, 2],
                                            [130, 2], [65, 2]]))
                            jd, base = [512, 2], 0
                            rjd = [4, 2]
                        else:
                            rc = recp.tile([128, 4], F32, tag="rc")
                            nc.vector.reciprocal(
                                rc,
                                bass.AP(tensor=avt.tensor,
                                        offset=avt.offset + bank * 512 + 64,
                                        ap=[list(avt.ap[0]), [130, 2],
                                            [65, 2]]))
                            jd, base = [130, 2], bank * 512
                            rjd = [2, 2]
                        nj = 4 if bank is None else 2
                        j0 = 0 if bank is None else 2 * bank
                        for par in range(2):
                            if bank is None:
                                dstd = [[1024, 2], [512, 2]]
                                srcd = [[512, 2], [130, 2]]
                                rcd = [[4, 2], [2, 2]]
                            else:
                                dstd = [[512, 2]]
                                srcd = [[130, 2]]
                                rcd = [[2, 2]]
                            dst = bass.AP(
                                tensor=attn.tensor,
                                offset=attn.offset + (4 * qt + j0) * 512
                                + (hp * 2 + par) * 64,
                                ap=[list(attn.ap[0])] + dstd + [[1, 64]])
                            src = bass.AP(
                                tensor=avt.tensor,
                                offset=avt.offset + base + par * 65,
                                ap=[list(avt.ap[0])] + srcd + [[1, 64]])
                            rcb = bass.AP(
                                tensor=rc.tensor, offset=rc.offset + par,
                                ap=[list(rc.ap[0])] + rcd + [[0, 64]])
                            nc.vector.tensor_tensor(
                                out=dst, in0=src, in1=rcb, op=AX.mult)

                    def xpose_tt(tt):
                        nc.sync.dma_start_transpose(
                            out=bass.AP(
                                tensor=attnT.tensor,
                                offset=attnT.offset + tt * 128,
                                ap=[list(attnT.ap[0]), [T, 4], [1, 128]]),
                            in_=attn[:, tt * 512:(tt + 1) * 512])

                    def emit_attnT(qt):
                        for j in range(4):
                            xpose_tt(4 * qt + j)

                    def out_chain(mt, n):
                        ps = pjpsum.tile([128, 512], F32, tag="pj", name="y")
                        for kk in range(4):
                            nc.tensor.matmul(
                                ps,
                                lhsT=attnT[:, kk * T + mt * 128:
                                           kk * T + (mt + 1) * 128],
                                rhs=woutb[:, kk * D + n * 512:
                                          kk * D + (n + 1) * 512],
                                start=(kk == 0), stop=(kk == 3))
                        yt = yo.tile([128, 512], F32, tag="yt", name="yt")
                        nc.vector.tensor_copy(yt, ps)
                        nc.sync.dma_start(
                            out=out_ext[mt * 128:(mt + 1) * 128,
                                        n * 512:(n + 1) * 512], in_=yt)

                    def emit_outproj(mts):
                        for mt in mts:
                            for n in range(2):
                                out_chain(mt, n)

                    from collections import deque

                    # Projection chains for head-pair hp+1 (and, on the last
                    # row, out-proj chains) are drained as fillers between an
                    # attention chunk's exp and its AV matmuls, so the PE has
                    # exp-independent work exactly where it would stall.
                    for hp in range(HLOC // 2):
                        fillers = deque()
                        if hp < 3:
                            for sec in (0, 1):
                                for n in range(NQT):
                                    fillers.append(
                                        (lambda m, s, nn:
                                         lambda: qk_chain(m, s, nn))(
                                             hp + 1, sec, n))
                        for qt in range(NQT):
                            if hp == 0:
                                if qt == 0:
                                    emit_vproj(range(0, 4))
                                    emit_qkproj(0, ns=[0])
                                else:
                                    emit_qkproj(0, ns=[qt])
                                    emit_vproj(range(4 * qt, 4 * qt + 4))
                            if hp == 3 and qt >= 1:
                                for mt in range(4 * (qt - 1), 4 * qt):
                                    for n in range(2):
                                        fillers.append(
                                            (lambda m, nn:
                                             lambda: out_chain(m, nn))(mt, n))
                            tail = hp == 3 and qt == 3
                            emit_unit(hp, qt, fillers,
                                      per_slot=(2 if hp == 3 else
                                                2 if qt >= 2 else 0),
                                      tail=tail)
                            if hp == 3 and not tail:
                                emit_attnT(qt)
                        if hp < 3:
                            while fillers:
                                fillers.popleft()()
                    emit_outproj(range(12, 16))
                    while fillers:
                        fillers.popleft()()
                    if DEBUG_DUMP:
                        with tc.tile_pool(name="dbgp", bufs=2) as dbgp:
                            for nm, src in (("xT", xT), ("qT", qT), ("kT", kT),
                                            ("vaug", vaug), ("attn", attn),
                                            ("attnT", attnT)):
                                w = src.shape[1]
                                for c0 in range(0, w, 2048):
                                    cw = min(2048, w - c0)
                                    t = dbgp.tile([128, cw], F32, tag="dbg")
                                    nc.vector.tensor_copy(t, src[:, c0:c0 + cw])
                                    nc.sync.dma_start(
                                        out=dbg[nm][0:128, c0:c0 + cw], in_=t)

    _split_multiwaits(nc)
    return nc


_NC_CACHE = {}


def get_nc():
    if "nc" not in _NC_CACHE:
        _NC_CACHE["nc"] = build_nc()
    return _NC_CACHE["nc"]


def make_in_maps(x, W_qkv, b_qkv, W_out):
    in_maps = []
    for c in range(8):
        b, g = c // 2, c % 2
        s = slice(512 * g, 512 * (g + 1))
        wslice = np.concatenate(
            [W_qkv[:, 512 * g:512 * (g + 1)],
             W_qkv[:, 1024 + 512 * g:1024 + 512 * (g + 1)],
             W_qkv[:, 2048 + 512 * g:2048 + 512 * (g + 1)]], axis=1)
        bslice = np.concatenate(
            [b_qkv[512 * g:512 * (g + 1)],
             b_qkv[1024 + 512 * g:1024 + 512 * (g + 1)],
             b_qkv[2048 + 512 * g:2048 + 512 * (g + 1)]])
        in_maps.append({
            "x": np.ascontiguousarray(x[b], dtype=np.float32),
            "W_qkv": np.ascontiguousarray(wslice, dtype=np.float32),
            "b_qkv": np.ascontiguousarray(bslice, dtype=np.float32),
            "W_out": np.ascontiguousarray(W_out[s], dtype=np.float32),
        })
    return in_maps


def kernel(x, W_qkv, b_qkv, W_out, b_out):
    x = np.asarray(x)
    W_qkv = np.asarray(W_qkv)
    b_qkv = np.asarray(b_qkv)
    W_out = np.asarray(W_out)
    b_out = np.asarray(b_out)
    nc = get_nc()
    in_maps = make_in_maps(x, W_qkv, b_qkv, W_out)
    res = run_bass_kernel_spmd(nc, in_maps, core_ids=list(range(8))).results
    out = np.stack(
        [res[2 * b]["out"] + res[2 * b + 1]["out"] for b in range(4)], axis=0)
    out = out + b_out[None, None, :]
    return out.astype(np.float32)


# revision 55
# speedup vs baseline: 1.2621x; 1.0021x over previous
"""Causal self-attention (B=4, T=2048, D=1024, H=16) on 8 TRN2 NeuronCores.

Sharding: core c -> (batch b = c//2, head-group g = c%2 of 8 heads).
Each core computes QKV projection for its 8 heads, causal attention, and a
partial out-projection (its heads' rows of W_out). The two partials per batch
are summed on the host during unshard (the "all-reduce after out_proj" of the
tensor-parallel scheme).

Per-core program (identical SPMD on all 8 cores):
  1. Loads: gpsimd (SWDGE) casting DMAs bring x, W_qkv (V columns first),
     and W_out from DRAM f32 into SBUF bf16 directly. xT is built by bf16
     PE transposes (1 cycle/row) into a bf16 PSUM bank, then one merged
     VectorE copy per t-tile (2-byte 2x mode).
  2. qT/kT = W^T-stationary matmuls -> [512(feat), 2048(t)] bf16 with the
     bias added during the PSUM->SBUF move on VectorE; V = xT-stationary
     matmuls -> [2048(t), 8 heads x 65] bf16 augmented with a ones column
     per head (softmax denominator via the same AV matmul).
  3. Per (head-pair, q-tile of 512): scoresT [k,q] in PSUM (2 heads packed
     into partition halves of the PE), exp on ScalarE (scale=1/8, fp32 in ->
     bf16 out, live query sub-ranges only), causal 0/1-mask multiply on the
     four diagonal 128x128 blocks only (VectorE). AV runs in the flipped
     orientation out[q(128 part), 65(free)] = P[k,qsub]^T @ Vh[k,65], one
     PSUM-accumulated chain per (q-subtile, head) packed 4-per-bank in a
     single 2-bank accumulator tile (one start / one stop per 2KB zero
     region; sibling chains ride on the bank-wide zero). Dead (k>q)
     subtiles are skipped entirely: 65 PE rows per (ktile, head, qsub)
     instead of 512 in the [65 part, q free] orientation. Normalization is
     a batched VectorE reciprocal of the denominator columns plus stride-0
     broadcast multiplies into attn [t, dloc] bf16.
  4. attn chunks are DMA-xbar-transposed (SP queue) into attnT [dloc, t];
     out_proj matmuls (attnT-stationary vs W_out rows) -> PSUM -> VectorE
     copy -> DRAM. Emitted one q-tile behind the last head-pair.

Scheduling: projection chains for head-pair hp+1 (and out-proj chains on
the last row) are drained as fillers inside the attention chunks exactly
where the PE would stall on exp latency; the last unit finishes its two
accumulator banks separately so normalize/transpose/out-proj of the first
half overlap the second half's AV matmuls.
"""

import numpy as np

import concourse.bass as bass
import concourse.mybir as mybir
import concourse.tile as tile
from concourse.bass_utils import run_bass_kernel_spmd

DEBUG_DUMP = False

F32 = mybir.dt.float32
BF16 = mybir.dt.bfloat16
AX = mybir.AluOpType

T = 2048
D = 1024
HLOC = 8          # heads per core
DKH = 64
QT = 512          # query tile
NQT = T // QT     # 4
NDIN = D // 128   # 8
NMT = 4           # q/k feature m-tiles (512 local feats / 128)
VA = 65           # V cols per head incl. ones column
NTT = T // 128    # 16
EXP = mybir.ActivationFunctionType.Exp


_NOP_ID = [0]


def _split_multiwaits(nc, limit=1):
    """This toolchain's walrus rejects more than one sync-wait on an
    instruction ("Too many sync wait commands"), and rejects any sync-wait
    on the DMA-xpose descriptor. Move excess waits onto same-engine NOPs
    inserted immediately before the instruction — the engine sequencer
    executes them in program order, so semantics are preserved."""
    for f in nc.m.functions:
        for blk in f.blocks:
            new = []
            changed = False
            for inst in blk.instructions:
                si = inst.sync_info
                lim = 0 if type(inst).__name__ == "InstDmaTransposeAnt" else limit
                if si is not None and len(si.on_wait) > lim:
                    waits = list(si.on_wait)
                    inst.sync_info = mybir.SyncInfo(
                        on_wait=waits[:lim], on_update=list(si.on_update))
                    for w in waits[lim:]:
                        _NOP_ID[0] += 1
                        nop = mybir.InstNoOp(
                            name=f"waitnop-{_NOP_ID[0]}", ins=[], outs=[])
                        nop.engine = inst.engine
                        nop.sync_info = mybir.SyncInfo(on_wait=[w], on_update=[])
                        new.append(nop)
                    changed = True
                new.append(inst)
            if changed:
                blk.instructions = new


def build_nc():
    nc = bass.Bass()
    x_ext = nc.declare_dram_parameter("x", [T, D], F32, isOutput=False)
    w_ext = nc.declare_dram_parameter("W_qkv", [D, 3 * 512], F32, isOutput=False)
    b_ext = nc.declare_dram_parameter("b_qkv", [3 * 512], F32, isOutput=False)
    wo_ext = nc.declare_dram_parameter("W_out", [512, D], F32, isOutput=False)
    out_ext = nc.declare_dram_parameter("out", [T, D], F32, isOutput=True)
    if DEBUG_DUMP:
        dbg = {
            "xT": nc.declare_dram_parameter("d_xT", [128, NDIN * T], F32, isOutput=True),
            "qT": nc.declare_dram_parameter("d_qT", [128, NMT * T], F32, isOutput=True),
            "kT": nc.declare_dram_parameter("d_kT", [128, NMT * T], F32, isOutput=True),
            "vaug": nc.declare_dram_parameter("d_vaug", [128, NTT * HLOC * VA], F32, isOutput=True),
            "attn": nc.declare_dram_parameter("d_attn", [128, NTT * 512], F32, isOutput=True),
            "attnT": nc.declare_dram_parameter("d_attnT", [128, 4 * T], F32, isOutput=True),
        }

    with tile.TileContext(nc) as tc:
        with (
            tc.tile_pool(name="const", bufs=1) as constp,
            tc.tile_pool(name="big", bufs=1) as bigp,
        ):
            # causal 0/1 mask [128, 2x512] bf16 (head-duplicated triangle):
            # mask[p, (h, f)] = 1 if f >= p else 0. Diagonal 128x128 blocks
            # use the [:, :, 0:128] slice. Generated AFTER the first load
            # DMAs are issued (these ops run on Pool and would delay the
            # SWDGE descriptor generation of the x/W casting loads).
            maskt = constp.tile([128, 1024], BF16, tag="maskt")

            def emit_mask_gen():
                nc.gpsimd.memset(maskt, 1.0)
                mk3 = maskt.rearrange("p (h f) -> p h f", f=512)
                nc.gpsimd.affine_select(
                    out=mk3, in_=mk3,
                    compare_op=AX.is_ge, fill=0.0,
                    base=0, channel_multiplier=-1,
                    pattern=[[0, 2], [1, 512]],
                )

            # biases: per-partition vectors for q/k feature tiles, broadcast
            # tile for V (bias along the free dv axis); DMAs issued inside
            # the load phase (after the first x tiles) to keep the DMA
            # device free for the critical-path loads.
            bq_sb = constp.tile([128, NMT], F32, tag="bq")
            bk_sb = constp.tile([128, NMT], F32, tag="bk")
            bv_sb = constp.tile([128, 512], F32, tag="bv")

            def emit_bias_loads():
                nc.sync.dma_start(
                    out=bq_sb,
                    in_=b_ext[0:512].rearrange("(m p) -> p m", p=128))
                nc.sync.dma_start(
                    out=bk_sb,
                    in_=b_ext[512:1024].rearrange("(m p) -> p m", p=128))
                bv_src = b_ext[1024:1536]
                nc.sync.dma_start(
                    out=bv_sb,
                    in_=bass.AP(tensor=bv_src.tensor, offset=bv_src.offset,
                                ap=[[0, 128]] + list(bv_src.ap)),
                )

            # persistent activations / weights (all bf16)
            qT = bigp.tile([128, NMT * T], BF16, tag="qT")
            kT = bigp.tile([128, NMT * T], BF16, tag="kT")
            vaug = bigp.tile([128, NTT * HLOC * VA], BF16, tag="vaug")
            attn = bigp.tile([128, NTT * 512], BF16, tag="attn")
            attnT = bigp.tile([128, 4 * T], BF16, tag="attnT")
            woutb = bigp.tile([128, 4 * D], BF16, tag="woutb")
            wbf = bigp.tile([128, NDIN * 1536], BF16, tag="wbf")
            xT = bigp.tile([128, NDIN * T], BF16, tag="xT")

            with (
                tc.tile_pool(name="pjpsum", bufs=2, space="PSUM") as pjpsum,
            ):
                # ---- loads: gpsimd casting DMAs (f32->bf16 in flight,
                # batched 4 row-chunks per DMA to amortize SWDGE gen) +
                # ACT-issued DMA xbar-transposes for xT
                def cast_load(dst, dst_off, src, row0, nrow, width,
                              dst_stride=None, src_roww=None, src_col0=0):
                    # f32->bf16 casting DMA of `nrow` 128-row chunks; the
                    # source may be a column slice [src_col0, src_col0+width)
                    # of rows with full width src_roww.
                    sw = src_roww or width
                    ds = dst_stride or width
                    nc.gpsimd.dma_start(
                        out=bass.AP(
                            tensor=dst.tensor, offset=dst.offset + dst_off,
                            ap=[list(dst.ap[0]), [ds, nrow], [1, width]]),
                        in_=bass.AP(
                            tensor=src.tensor,
                            offset=src.offset + row0 * sw + src_col0,
                            ap=[[sw, 128], [128 * sw, nrow], [1, width]]),
                    )

                # 2 row-chunks (256 descriptors) per DMA so 4 fit in the
                # SWDGE descriptor ring; x group 0 first (the xT transposes
                # are on the critical path), weights interleaved behind it.
                # x: Pool casting DMAs (f32->bf16 in flight, 4 t-tiles per
                # DMA) -> bf16 PE transposes (1 cycle/row) into a bf16 PSUM
                # bank -> one merged DVE copy per t-tile (2-byte 2x mode).
                # W: Pool casting DMAs, V columns first (vproj needs only
                # those), interleaved behind x group 0.
                from concourse.masks import make_identity
                ident = constp.tile([128, 128], BF16, tag="ident")
                with (
                    tc.tile_pool(name="xstage", bufs=4) as xstage,
                    tc.tile_pool(name="tpsum", bufs=2, space="PSUM") as tpsum,
                ):
                    def load_xg(g, nrow=4, row0=None):
                        xg = xstage.tile([128, nrow * D], BF16, tag="xg")
                        cast_load(xg, 0, x_ext[0:1, :],
                                  512 * g if row0 is None else row0, nrow, D)
                        return xg

                    def xpose_x(xg, r):
                        tp = tpsum.tile([128, 1024], BF16, tag="tp")
                        for dj in range(NDIN):
                            nc.tensor.transpose(
                                tp[:, dj * 128:(dj + 1) * 128],
                                xg[:, r * D + dj * 128:r * D + (dj + 1) * 128],
                                ident)
                        return tp

                    def xpose_tts(xg, g):
                        for r in range(4):
                            tp = xpose_x(xg, r)
                            nc.vector.tensor_copy(
                                bass.AP(tensor=xT.tensor,
                                        offset=xT.offset + (4 * g + r) * 128,
                                        ap=[list(xT.ap[0]), [T, NDIN],
                                            [1, 128]]),
                                tp.rearrange("p (dj t) -> p dj t", t=128))

                    xg0a = load_xg(0, nrow=1, row0=0)
                    cast_load(wbf, 1024, w_ext[0:1, :], 0, 4, 512,
                              dst_stride=1536, src_roww=1536, src_col0=1024)
                    xg0b = load_xg(0, nrow=3, row0=128)
                    cast_load(wbf, 4 * 1536 + 1024, w_ext[0:1, :], 512, 4, 512,
                              dst_stride=1536, src_roww=1536, src_col0=1024)
                    make_identity(nc, ident)
                    emit_mask_gen()
                    tp = xpose_x(xg0a, 0)
                    nc.vector.tensor_copy(
                        bass.AP(tensor=xT.tensor, offset=xT.offset,
                                ap=[list(xT.ap[0]), [T, NDIN], [1, 128]]),
                        tp.rearrange("p (dj t) -> p dj t", t=128))
                    for r in range(3):
                        tp = xpose_x(xg0b, r)
                        nc.vector.tensor_copy(
                            bass.AP(tensor=xT.tensor,
                                    offset=xT.offset + (1 + r) * 128,
                                    ap=[list(xT.ap[0]), [T, NDIN], [1, 128]]),
                            tp.rearrange("p (dj t) -> p dj t", t=128))
                    cast_load(wbf, 0, w_ext[0:1, :], 0, 4, 1024,
                              dst_stride=1536, src_roww=1536, src_col0=0)
                    cast_load(wbf, 4 * 1536, w_ext[0:1, :], 512, 4, 1024,
                              dst_stride=1536, src_roww=1536, src_col0=0)
                    emit_bias_loads()
                    xg1 = load_xg(1)
                    xpose_tts(xg1, 1)
                    xg2 = load_xg(2)
                    xpose_tts(xg2, 2)
                    cast_load(woutb, 0, wo_ext[0:1, :], 0, 4, D)
                    xg3 = load_xg(3)
                    xpose_tts(xg3, 3)

                def qk_chain(mt, sec, n):
                    dst, bias = (qT, bq_sb) if sec == 0 else (kT, bk_sb)
                    ps = pjpsum.tile([128, 512], F32, tag="pj")
                    for kk in range(NDIN):
                        nc.tensor.matmul(
                            ps,
                            lhsT=wbf[:, kk * 1536 + sec * 512 + mt * 128:
                                     kk * 1536 + sec * 512 + (mt + 1) * 128],
                            rhs=xT[:, kk * T + n * 512: kk * T + (n + 1) * 512],
                            start=(kk == 0), stop=(kk == NDIN - 1),
                        )
                    nc.vector.tensor_scalar(
                        out=dst[:, mt * T + n * 512: mt * T + (n + 1) * 512],
                        in0=ps, scalar1=bias[:, mt:mt + 1], scalar2=None,
                        op0=AX.add)

                def emit_qkproj(mt, ns=None):
                    for sec in (0, 1):
                        for n in (range(NQT) if ns is None else ns):
                            qk_chain(mt, sec, n)

                def emit_vproj(tts):
                    for tt in tts:
                        ps = pjpsum.tile([128, 512], F32, tag="pj")
                        for kk in range(NDIN):
                            nc.tensor.matmul(
                                ps,
                                lhsT=xT[:, kk * T + tt * 128: kk * T + (tt + 1) * 128],
                                rhs=wbf[:, kk * 1536 + 1024: kk * 1536 + 1536],
                                start=(kk == 0), stop=(kk == NDIN - 1),
                            )
                        blk = vaug[:, tt * (HLOC * VA):(tt + 1) * (HLOC * VA)]
                        blk3 = blk.rearrange("p (h c) -> p h c", c=VA)
                        nc.vector.tensor_tensor(
                            out=blk3[:, :, 0:64],
                            in0=ps.rearrange("p (h c) -> p h c", c=64),
                            in1=bv_sb.rearrange("p (h c) -> p h c", c=64),
                            op=AX.add)
                        nc.vector.memset(blk3[:, :, 64:65], 1.0)

                with (
                    tc.tile_pool(name="scps", bufs=2, space="PSUM") as scps,
                    tc.tile_pool(name="avps", bufs=1, space="PSUM") as avps,
                    tc.tile_pool(name="ptp", bufs=10) as ptp,
                    tc.tile_pool(name="recp", bufs=2) as recp,
                    tc.tile_pool(name="yo", bufs=8) as yo,
                ):
                    m3 = maskt.rearrange("p (h q) -> p h q", q=512)

                    def av_off(j, par):
                        s = 2 * j + par
                        return (s // 4) * 512 + (s % 4) * 65

                    def emit_unit(hp, qt, fillers=None, per_slot=1,
                                  tail=False):
                        def fill(k):
                            for _ in range(k):
                                if fillers:
                                    fillers.popleft()()

                        def av_mm(kt, j, par, pt_t):
                            h = 2 * hp + par
                            off = av_off(j, par)
                            # PSUM start/stop are per 2KB bank (zero
                            # region): exactly one start (zeroes the bank)
                            # and one stop per bank; sibling chains ride.
                            nc.tensor.matmul(
                                avt[:, off:off + VA],
                                lhsT=pt_t[:, par * 512 + j * 128:
                                          par * 512 + (j + 1) * 128],
                                rhs=vaug[:, kt * (HLOC * VA) + h * VA:
                                         kt * (HLOC * VA) + (h + 1) * VA],
                                start=(kt == 0 and par == 0 and j % 2 == 0),
                                stop=(par == 1 and j % 2 == 1
                                      and kt == 4 * qt + j),
                            )

                        nkt = 4 * (qt + 1)
                        avt = avps.tile([128, 1024], F32, tag="av")
                        for c0 in range(0, nkt, 8):
                            is_last = tail and c0 + 8 >= nkt
                            chunk = list(range(c0, min(c0 + 8, nkt)))
                            pts = {}
                            for kt in chunk:
                                # diagonal k-tile (i >= 0): only queries
                                # f >= 128*i are live
                                i = kt - (nkt - 4)
                                lo_q = max(0, 128 * i)
                                ps = scps.tile([128, 1024], F32, tag="sc")
                                for par in range(2):
                                    lo, hi = par * 64, par * 64 + 64
                                    nc.tensor.matmul(
                                        ps[:, par * 512 + lo_q:(par + 1) * 512],
                                        lhsT=kT[lo:hi, hp * T + kt * 128:
                                                hp * T + (kt + 1) * 128],
                                        rhs=qT[lo:hi, hp * T + qt * 512 + lo_q:
                                               hp * T + (qt + 1) * 512],
                                        start=True, stop=True,
                                    )
                                pt_t = ptp.tile([128, 1024], BF16, tag="pt")
                                ps3 = ps.rearrange("p (h q) -> p h q", q=512)
                                pt3 = pt_t.rearrange("p (h q) -> p h q", q=512)
                                nc.scalar.activation(
                                    pt3[:, :, lo_q:512], ps3[:, :, lo_q:512],
                                    EXP, bias=0.0, scale=0.125)
                                if i >= 0:
                                    # mask the diagonal 128x128 block (both
                                    # heads): q-subtile j == i
                                    blk = bass.AP(
                                        tensor=pt_t.tensor,
                                        offset=pt_t.offset + lo_q,
                                        ap=[list(pt_t.ap[0]), [512, 2], [1, 128]])
                                    mblk = bass.AP(
                                        tensor=maskt.tensor,
                                        offset=maskt.offset,
                                        ap=[list(maskt.ap[0]), [512, 2], [1, 128]])
                                    nc.vector.tensor_tensor(
                                        out=blk, in0=blk, in1=mblk, op=AX.mult)
                                pts[kt] = pt_t
                            fill(1 if is_last else per_slot)
                            if not is_last:
                                for kt in chunk:
                                    for j in range(4):
                                        if kt > 4 * qt + j:
                                            continue  # fully-dead block
                                        for par in range(2):
                                            av_mm(kt, j, par, pts[kt])
                            else:
                                # tail: finish bank 0 (q-subtiles 0,1) first
                                # so its normalize / attnT transpose /
                                # out-proj overlap bank 1's AV matmuls;
                                # leftover filler chains are drained right
                                # after each bank's transposes to cover the
                                # transpose latency before out-proj starts
                                for bank in range(2):
                                    for kt in chunk:
                                        for j in (2 * bank, 2 * bank + 1):
                                            if kt > 4 * qt + j:
                                                continue
                                            for par in range(2):
                                                av_mm(kt, j, par, pts[kt])
                                    normalize(hp, qt, avt, bank=bank)
                                    xpose_tt(4 * qt + 2 * bank)
                                    xpose_tt(4 * qt + 2 * bank + 1)
                                    fill(3)
                        if not tail:
                            normalize(hp, qt, avt)

                    def normalize(hp, qt, avt, bank=None):
                        # rc[p, s] = 1 / denom(slot s); attn chunk cols
                        # (hp, par) of t-chunks 4qt..4qt+3 (or one bank's 2)
                        if bank is None:
                            rc = recp.tile([128, 8], F32, tag="rc")
                            nc.vector.reciprocal(
                                rc,
                                bass.AP(tensor=avt.tensor,
                                        offset=avt.offset + 64,
                                        ap=[list(avt.ap[0]), [512, 2],
                                            [130, 2], [65, 2]]))
                            jd, base = [512, 2], 0
                            rjd = [4, 2]
                        else:
                            rc = recp.tile([128, 4], F32, tag="rc")
                            nc.vector.reciprocal(
                                rc,
                                bass.AP(tensor=avt.tensor,
                                        offset=avt.offset + bank * 512 + 64,
                                        ap=[list(avt.ap[0]), [130, 2],
                                            [65, 2]]))
                            jd, base = [130, 2], bank * 512
                            rjd = [2, 2]
                        nj = 4 if bank is None else 2
                        j0 = 0 if bank is None else 2 * bank
                        for par in range(2):
                            if bank is None:
                                dstd = [[1024, 2], [512, 2]]
                                srcd = [[512, 2], [130, 2]]
                                rcd = [[4, 2], [2, 2]]
                            else:
                                dstd = [[512, 2]]
                                srcd = [[130, 2]]
                                rcd = [[2, 2]]
                            dst = bass.AP(
                                tensor=attn.tensor,
                                offset=attn.offset + (4 * qt + j0) * 512
                                + (hp * 2 + par) * 64,
                                ap=[list(attn.ap[0])] + dstd + [[1, 64]])
                            src = bass.AP(
                                tensor=avt.tensor,
                                offset=avt.offset + base + par * 65,
                                ap=[list(avt.ap[0])] + srcd + [[1, 64]])
                            rcb = bass.AP(
                                tensor=rc.tensor, offset=rc.offset + par,
                                ap=[list(rc.ap[0])] + rcd + [[0, 64]])
                            nc.vector.tensor_tensor(
                                out=dst, in0=src, in1=rcb, op=AX.mult)

                    def xpose_tt(tt):
                        nc.sync.dma_start_transpose(
                            out=bass.AP(
                                tensor=attnT.tensor,
                                offset=attnT.offset + tt * 128,
                                ap=[list(attnT.ap[0]), [T, 4], [1, 128]]),
                            in_=attn[:, tt * 512:(tt + 1) * 512])

                    def emit_attnT(qt):
                        for j in range(4):
                            xpose_tt(4 * qt + j)

                    def out_chain(mt, n):
                        ps = pjpsum.tile([128, 512], F32, tag="pj", name="y")
                        for kk in range(4):
                            nc.tensor.matmul(
                                ps,
                                lhsT=attnT[:, kk * T + mt * 128:
                                           kk * T + (mt + 1) * 128],
                                rhs=woutb[:, kk * D + n * 512:
                                          kk * D + (n + 1) * 512],
                                start=(kk == 0), stop=(kk == 3))
                        yt = yo.tile([128, 512], F32, tag="yt", name="yt")
                        nc.vector.tensor_copy(yt, ps)
                        nc.sync.dma_start(
                            out=out_ext[mt * 128:(mt + 1) * 128,
                                        n * 512:(n + 1) * 512], in_=yt)

                    def emit_outproj(mts):
                        for mt in mts:
                            for n in range(2):
                                out_chain(mt, n)

                    from collections import deque

                    # Projection chains for head-pair hp+1 (and, on the last
                    # row, out-proj chains) are drained as fillers between an
                    # attention chunk's exp and its AV matmuls, so the PE has
                    # exp-independent work exactly where it would stall.
                    for hp in range(HLOC // 2):
                        fillers = deque()
                        if hp < 3:
                            for sec in (0, 1):
                                for n in range(NQT):
                                    fillers.append(
                                        (lambda m, s, nn:
                                         lambda: qk_chain(m, s, nn))(
                                             hp + 1, sec, n))
                        for qt in range(NQT):
                            if hp == 0:
                                if qt == 0:
                                    emit_vproj(range(0, 4))
                                    emit_qkproj(0, ns=[0])
                                else:
                                    emit_qkproj(0, ns=[qt])
                                    emit_vproj(range(4 * qt, 4 * qt + 4))
                            if hp == 3 and qt >= 1:
                                for mt in range(4 * (qt - 1), 4 * qt):
                                    for n in range(2):
                                        fillers.append(
                                            (lambda m, nn:
                                             lambda: out_chain(m, nn))(mt, n))
                            tail = hp == 3 and qt == 3
                            emit_unit(hp, qt, fillers,
                                      per_slot=(2 if hp == 3 else
                                                2 if qt >= 2 else 0),
                                      tail=tail)
                            if hp == 3 and not tail:
                                emit_attnT(qt)
                        if hp < 3:
                            while fillers:
                                fillers.popleft()()
                    emit_outproj(range(12, 16))
                    while fillers:
                        fillers.popleft()()
                    if DEBUG_DUMP:
                        with tc.tile_pool(name="dbgp", bufs=2) as dbgp:
                            for nm, src in (("xT", xT), ("qT", qT), ("kT", kT),
                                            ("vaug", vaug), ("attn", attn),
                                            ("attnT", attnT)):
                                w = src.shape[1]
                                for c0 in range(0, w, 2048):
                                    cw = min(2048, w - c0)
                                    t = dbgp.tile([128, cw], F32, tag="dbg")
                                    nc.vector.tensor_copy(t, src[:, c0:c0 + cw])
                                    nc.sync.dma_start(
                                        out=dbg[nm][0:128, c0:c0 + cw], in_=t)

    _split_multiwaits(nc)
    return nc


_NC_CACHE = {}


def get_nc():
    if "nc" not in _NC_CACHE:
        _NC_CACHE["nc"] = build_nc()
    return _NC_CACHE["nc"]


def make_in_maps(x, W_qkv, b_qkv, W_out):
    in_maps = []
    for c in range(8):
        b, g = c // 2, c % 2
        s = slice(512 * g, 512 * (g + 1))
        wslice = np.concatenate(
            [W_qkv[:, 512 * g:512 * (g + 1)],
             W_qkv[:, 1024 + 512 * g:1024 + 512 * (g + 1)],
             W_qkv[:, 2048 + 512 * g:2048 + 512 * (g + 1)]], axis=1)
        bslice = np.concatenate(
            [b_qkv[512 * g:512 * (g + 1)],
             b_qkv[1024 + 512 * g:1024 + 512 * (g + 1)],
             b_qkv[2048 + 512 * g:2048 + 512 * (g + 1)]])
        in_maps.append({
            "x": np.ascontiguousarray(x[b], dtype=np.float32),
            "W_qkv": np.ascontiguousarray(wslice, dtype=np.float32),
            "b_qkv": np.ascontiguousarray(bslice, dtype=np.float32),
            "W_out": np.ascontiguousarray(W_out[s], dtype=np.float32),
        })
    return in_maps


def kernel(x, W_qkv, b_qkv, W_out, b_out):
    x = np.asarray(x)
    W_qkv = np.asarray(W_qkv)
    b_qkv = np.asarray(b_qkv)
    W_out = np.asarray(W_out)
    b_out = np.asarray(b_out)
    nc = get_nc()
    in_maps = make_in_maps(x, W_qkv, b_qkv, W_out)
    res = run_bass_kernel_spmd(nc, in_maps, core_ids=list(range(8))).results
    out = np.stack(
        [res[2 * b]["out"] + res[2 * b + 1]["out"] for b in range(4)], axis=0)
    out = out + b_out[None, None, :]
    return out.astype(np.float32)


# revision 63
# speedup vs baseline: 1.2642x; 1.0017x over previous
"""Causal self-attention (B=4, T=2048, D=1024, H=16) on 8 TRN2 NeuronCores.

Sharding: core c -> (batch b = c//2, head-group g = c%2 of 8 heads).
Each core computes QKV projection for its 8 heads, causal attention, and a
partial out-projection (its heads' rows of W_out). The two partials per batch
are summed on the host during unshard (the "all-reduce after out_proj" of the
tensor-parallel scheme).

Per-core program (identical SPMD on all 8 cores):
  1. Loads: gpsimd (SWDGE) casting DMAs bring x, W_qkv (V columns first),
     and W_out from DRAM f32 into SBUF bf16 directly. xT is built by bf16
     PE transposes (1 cycle/row) into a bf16 PSUM bank, then one merged
     VectorE copy per t-tile (2-byte 2x mode).
  2. qT/kT = W^T-stationary matmuls -> [512(feat), 2048(t)] bf16 with the
     bias added during the PSUM->SBUF move on VectorE; V = xT-stationary
     matmuls -> [2048(t), 8 heads x 65] bf16 augmented with a ones column
     per head (softmax denominator via the same AV matmul).
  3. Per (head-pair, q-tile of 512): scoresT [k,q] in PSUM (2 heads packed
     into partition halves of the PE), exp on ScalarE (scale=1/8, fp32 in ->
     bf16 out, live query sub-ranges only), causal 0/1-mask multiply on the
     four diagonal 128x128 blocks only (VectorE). AV runs in the flipped
     orientation out[q(128 part), 65(free)] = P[k,qsub]^T @ Vh[k,65], one
     PSUM-accumulated chain per (q-subtile, head) packed 4-per-bank in a
     single 2-bank accumulator tile (one start / one stop per 2KB zero
     region; sibling chains ride on the bank-wide zero). Dead (k>q)
     subtiles are skipped entirely: 65 PE rows per (ktile, head, qsub)
     instead of 512 in the [65 part, q free] orientation. Normalization is
     a batched VectorE reciprocal of the denominator columns plus stride-0
     broadcast multiplies into attn [t, dloc] bf16.
  4. attn chunks are DMA-xbar-transposed (SP queue) into attnT [dloc, t];
     out_proj matmuls (attnT-stationary vs W_out rows) -> PSUM -> VectorE
     copy -> DRAM. Emitted one q-tile behind the last head-pair.

Scheduling: projection chains for head-pair hp+1 (and out-proj chains on
the last row) are drained as fillers inside the attention chunks exactly
where the PE would stall on exp latency; the last unit finishes its two
accumulator banks separately so normalize/transpose/out-proj of the first
half overlap the second half's AV matmuls.
"""

import numpy as np

import concourse.bass as bass
import concourse.mybir as mybir
import concourse.tile as tile
from concourse.bass_utils import run_bass_kernel_spmd

DEBUG_DUMP = False

F32 = mybir.dt.float32
BF16 = mybir.dt.bfloat16
AX = mybir.AluOpType

T = 2048
D = 1024
HLOC = 8          # heads per core
DKH = 64
QT = 512          # query tile
NQT = T // QT     # 4
NDIN = D // 128   # 8
NMT = 4           # q/k feature m-tiles (512 local feats / 128)
VA = 65           # V cols per head incl. ones column
NTT = T // 128    # 16
EXP = mybir.ActivationFunctionType.Exp


_NOP_ID = [0]


def _split_multiwaits(nc, limit=1):
    """This toolchain's walrus rejects more than one sync-wait on an
    instruction ("Too many sync wait commands"), and rejects any sync-wait
    on the DMA-xpose descriptor. Move excess waits onto same-engine NOPs
    inserted immediately before the instruction — the engine sequencer
    executes them in program order, so semantics are preserved."""
    for f in nc.m.functions:
        for blk in f.blocks:
            new = []
            changed = False
            for inst in blk.instructions:
                si = inst.sync_info
                lim = 0 if type(inst).__name__ == "InstDmaTransposeAnt" else limit
                if si is not None and len(si.on_wait) > lim:
                    waits = list(si.on_wait)
                    inst.sync_info = mybir.SyncInfo(
                        on_wait=waits[:lim], on_update=list(si.on_update))
                    for w in waits[lim:]:
                        _NOP_ID[0] += 1
                        nop = mybir.InstNoOp(
                            name=f"waitnop-{_NOP_ID[0]}", ins=[], outs=[])
                        nop.engine = inst.engine
                        nop.sync_info = mybir.SyncInfo(on_wait=[w], on_update=[])
                        new.append(nop)
                    changed = True
                new.append(inst)
            if changed:
                blk.instructions = new


def build_nc():
    nc = bass.Bass()
    x_ext = nc.declare_dram_parameter("x", [T, D], F32, isOutput=False)
    w_ext = nc.declare_dram_parameter("W_qkv", [D, 3 * 512], F32, isOutput=False)
    b_ext = nc.declare_dram_parameter("b_qkv", [3 * 512], F32, isOutput=False)
    wo_ext = nc.declare_dram_parameter("W_out", [512, D], F32, isOutput=False)
    out_ext = nc.declare_dram_parameter("out", [T, D], F32, isOutput=True)
    if DEBUG_DUMP:
        dbg = {
            "xT": nc.declare_dram_parameter("d_xT", [128, NDIN * T], F32, isOutput=True),
            "qT": nc.declare_dram_parameter("d_qT", [128, NMT * T], F32, isOutput=True),
            "kT": nc.declare_dram_parameter("d_kT", [128, NMT * T], F32, isOutput=True),
            "vaug": nc.declare_dram_parameter("d_vaug", [128, NTT * HLOC * VA], F32, isOutput=True),
            "attn": nc.declare_dram_parameter("d_attn", [128, NTT * 512], F32, isOutput=True),
            "attnT": nc.declare_dram_parameter("d_attnT", [128, 4 * T], F32, isOutput=True),
        }

    with tile.TileContext(nc) as tc:
        with (
            tc.tile_pool(name="const", bufs=1) as constp,
            tc.tile_pool(name="big", bufs=1) as bigp,
        ):
            # causal 0/1 mask [128, 2x512] bf16 (head-duplicated triangle):
            # mask[p, (h, f)] = 1 if f >= p else 0. Diagonal 128x128 blocks
            # use the [:, :, 0:128] slice. Generated AFTER the first load
            # DMAs are issued (these ops run on Pool and would delay the
            # SWDGE descriptor generation of the x/W casting loads).
            maskt = constp.tile([128, 1024], BF16, tag="maskt")

            def emit_mask_gen():
                nc.gpsimd.memset(maskt, 1.0)
                mk3 = maskt.rearrange("p (h f) -> p h f", f=512)
                nc.gpsimd.affine_select(
                    out=mk3, in_=mk3,
                    compare_op=AX.is_ge, fill=0.0,
                    base=0, channel_multiplier=-1,
                    pattern=[[0, 2], [1, 512]],
                )

            # biases: per-partition vectors for q/k feature tiles, broadcast
            # tile for V (bias along the free dv axis); DMAs issued inside
            # the load phase (after the first x tiles) to keep the DMA
            # device free for the critical-path loads.
            bq_sb = constp.tile([128, NMT], F32, tag="bq")
            bk_sb = constp.tile([128, NMT], F32, tag="bk")
            bv_sb = constp.tile([128, 512], F32, tag="bv")

            def emit_bias_loads():
                nc.sync.dma_start(
                    out=bq_sb,
                    in_=b_ext[0:512].rearrange("(m p) -> p m", p=128))
                nc.sync.dma_start(
                    out=bk_sb,
                    in_=b_ext[512:1024].rearrange("(m p) -> p m", p=128))
                bv_src = b_ext[1024:1536]
                nc.sync.dma_start(
                    out=bv_sb,
                    in_=bass.AP(tensor=bv_src.tensor, offset=bv_src.offset,
                                ap=[[0, 128]] + list(bv_src.ap)),
                )

            # persistent activations / weights (all bf16)
            qT = bigp.tile([128, NMT * T], BF16, tag="qT")
            kT = bigp.tile([128, NMT * T], BF16, tag="kT")
            vaug = bigp.tile([128, NTT * HLOC * VA], BF16, tag="vaug")
            attn = bigp.tile([128, NTT * 512], BF16, tag="attn")
            attnT = bigp.tile([128, 4 * T], BF16, tag="attnT")
            woutb = bigp.tile([128, 4 * D], BF16, tag="woutb")
            wbf = bigp.tile([128, NDIN * 1536], BF16, tag="wbf")
            xT = bigp.tile([128, NDIN * T], BF16, tag="xT")

            with (
                tc.tile_pool(name="pjpsum", bufs=2, space="PSUM") as pjpsum,
            ):
                # ---- loads: gpsimd casting DMAs (f32->bf16 in flight,
                # batched 4 row-chunks per DMA to amortize SWDGE gen) +
                # ACT-issued DMA xbar-transposes for xT
                def cast_load(dst, dst_off, src, row0, nrow, width,
                              dst_stride=None, src_roww=None, src_col0=0):
                    # f32->bf16 casting DMA of `nrow` 128-row chunks; the
                    # source may be a column slice [src_col0, src_col0+width)
                    # of rows with full width src_roww.
                    sw = src_roww or width
                    ds = dst_stride or width
                    nc.gpsimd.dma_start(
                        out=bass.AP(
                            tensor=dst.tensor, offset=dst.offset + dst_off,
                            ap=[list(dst.ap[0]), [ds, nrow], [1, width]]),
                        in_=bass.AP(
                            tensor=src.tensor,
                            offset=src.offset + row0 * sw + src_col0,
                            ap=[[sw, 128], [128 * sw, nrow], [1, width]]),
                    )

                # 2 row-chunks (256 descriptors) per DMA so 4 fit in the
                # SWDGE descriptor ring; x group 0 first (the xT transposes
                # are on the critical path), weights interleaved behind it.
                # x: Pool casting DMAs (f32->bf16 in flight, 4 t-tiles per
                # DMA) -> bf16 PE transposes (1 cycle/row) into a bf16 PSUM
                # bank -> one merged DVE copy per t-tile (2-byte 2x mode).
                # W: Pool casting DMAs, V columns first (vproj needs only
                # those), interleaved behind x group 0.
                from concourse.masks import make_identity
                ident = constp.tile([128, 128], BF16, tag="ident")
                with (
                    tc.tile_pool(name="xstage", bufs=4) as xstage,
                    tc.tile_pool(name="tpsum", bufs=2, space="PSUM") as tpsum,
                ):
                    def load_xg(g, nrow=4, row0=None):
                        xg = xstage.tile([128, nrow * D], BF16, tag="xg")
                        cast_load(xg, 0, x_ext[0:1, :],
                                  512 * g if row0 is None else row0, nrow, D)
                        return xg

                    def xpose_x(xg, r):
                        tp = tpsum.tile([128, 1024], BF16, tag="tp")
                        for dj in range(NDIN):
                            nc.tensor.transpose(
                                tp[:, dj * 128:(dj + 1) * 128],
                                xg[:, r * D + dj * 128:r * D + (dj + 1) * 128],
                                ident)
                        return tp

                    def xpose_tts(xg, g):
                        for r in range(4):
                            tp = xpose_x(xg, r)
                            nc.vector.tensor_copy(
                                bass.AP(tensor=xT.tensor,
                                        offset=xT.offset + (4 * g + r) * 128,
                                        ap=[list(xT.ap[0]), [T, NDIN],
                                            [1, 128]]),
                                tp.rearrange("p (dj t) -> p dj t", t=128))

                    xg0a = load_xg(0, nrow=1, row0=0)
                    cast_load(wbf, 1024, w_ext[0:1, :], 0, 4, 512,
                              dst_stride=1536, src_roww=1536, src_col0=1024)
                    xg0b = load_xg(0, nrow=3, row0=128)
                    cast_load(wbf, 4 * 1536 + 1024, w_ext[0:1, :], 512, 4, 512,
                              dst_stride=1536, src_roww=1536, src_col0=1024)
                    make_identity(nc, ident)
                    emit_mask_gen()
                    tp = xpose_x(xg0a, 0)
                    nc.vector.tensor_copy(
                        bass.AP(tensor=xT.tensor, offset=xT.offset,
                                ap=[list(xT.ap[0]), [T, NDIN], [1, 128]]),
                        tp.rearrange("p (dj t) -> p dj t", t=128))
                    for r in range(3):
                        tp = xpose_x(xg0b, r)
                        nc.vector.tensor_copy(
                            bass.AP(tensor=xT.tensor,
                                    offset=xT.offset + (1 + r) * 128,
                                    ap=[list(xT.ap[0]), [T, NDIN], [1, 128]]),
                            tp.rearrange("p (dj t) -> p dj t", t=128))
                    cast_load(wbf, 0, w_ext[0:1, :], 0, 4, 1024,
                              dst_stride=1536, src_roww=1536, src_col0=0)
                    cast_load(wbf, 4 * 1536, w_ext[0:1, :], 512, 4, 1024,
                              dst_stride=1536, src_roww=1536, src_col0=0)
                    emit_bias_loads()
                    xg1 = load_xg(1)
                    xpose_tts(xg1, 1)
                    xg2 = load_xg(2)
                    xpose_tts(xg2, 2)
                    cast_load(woutb, 0, wo_ext[0:1, :], 0, 4, D)
                    xg3 = load_xg(3)
                    xpose_tts(xg3, 3)

                def qk_chain(mt, sec, n):
                    dst, bias = (qT, bq_sb) if sec == 0 else (kT, bk_sb)
                    ps = pjpsum.tile([128, 512], F32, tag="pj")
                    for kk in range(NDIN):
                        nc.tensor.matmul(
                            ps,
                            lhsT=wbf[:, kk * 1536 + sec * 512 + mt * 128:
                                     kk * 1536 + sec * 512 + (mt + 1) * 128],
                            rhs=xT[:, kk * T + n * 512: kk * T + (n + 1) * 512],
                            start=(kk == 0), stop=(kk == NDIN - 1),
                        )
                    nc.vector.tensor_scalar(
                        out=dst[:, mt * T + n * 512: mt * T + (n + 1) * 512],
                        in0=ps, scalar1=bias[:, mt:mt + 1], scalar2=None,
                        op0=AX.add)

                def emit_qkproj(mt, ns=None):
                    for sec in (0, 1):
                        for n in (range(NQT) if ns is None else ns):
                            qk_chain(mt, sec, n)

                def emit_vproj(tts):
                    for tt in tts:
                        ps = pjpsum.tile([128, 512], F32, tag="pj")
                        for kk in range(NDIN):
                            nc.tensor.matmul(
                                ps,
                                lhsT=xT[:, kk * T + tt * 128: kk * T + (tt + 1) * 128],
                                rhs=wbf[:, kk * 1536 + 1024: kk * 1536 + 1536],
                                start=(kk == 0), stop=(kk == NDIN - 1),
                            )
                        blk = vaug[:, tt * (HLOC * VA):(tt + 1) * (HLOC * VA)]
                        blk3 = blk.rearrange("p (h c) -> p h c", c=VA)
                        nc.vector.tensor_tensor(
                            out=blk3[:, :, 0:64],
                            in0=ps.rearrange("p (h c) -> p h c", c=64),
                            in1=bv_sb.rearrange("p (h c) -> p h c", c=64),
                            op=AX.add)
                        nc.vector.memset(blk3[:, :, 64:65], 1.0)

                with (
                    tc.tile_pool(name="scps", bufs=2, space="PSUM") as scps,
                    tc.tile_pool(name="avps", bufs=1, space="PSUM") as avps,
                    tc.tile_pool(name="ptp", bufs=10) as ptp,
                    tc.tile_pool(name="recp", bufs=2) as recp,
                    tc.tile_pool(name="yo", bufs=8) as yo,
                ):
                    m3 = maskt.rearrange("p (h q) -> p h q", q=512)

                    def av_off(j, par):
                        s = 2 * j + par
                        return (s // 4) * 512 + (s % 4) * 65

                    def emit_unit(hp, qt, fillers=None, per_slot=1,
                                  tail=False):
                        def fill(k):
                            for _ in range(k):
                                if fillers:
                                    fillers.popleft()()

                        def av_mm(kt, j, par, pt_t):
                            h = 2 * hp + par
                            off = av_off(j, par)
                            # PSUM start/stop are per 2KB bank (zero
                            # region): exactly one start (zeroes the bank)
                            # and one stop per bank; sibling chains ride.
                            nc.tensor.matmul(
                                avt[:, off:off + VA],
                                lhsT=pt_t[:, par * 512 + j * 128:
                                          par * 512 + (j + 1) * 128],
                                rhs=vaug[:, kt * (HLOC * VA) + h * VA:
                                         kt * (HLOC * VA) + (h + 1) * VA],
                                start=(kt == 0 and par == 0 and j % 2 == 0),
                                stop=(par == 1 and j % 2 == 1
                                      and kt == 4 * qt + j),
                            )

                        nkt = 4 * (qt + 1)
                        avt = avps.tile([128, 1024], F32, tag="av")
                        for c0 in range(0, nkt, 8):
                            is_last = tail and c0 + 8 >= nkt
                            chunk = list(range(c0, min(c0 + 8, nkt)))
                            pts = {}
                            for kt in chunk:
                                # diagonal k-tile (i >= 0): only queries
                                # f >= 128*i are live
                                i = kt - (nkt - 4)
                                lo_q = max(0, 128 * i)
                                ps = scps.tile([128, 1024], F32, tag="sc")
                                for par in range(2):
                                    lo, hi = par * 64, par * 64 + 64
                                    nc.tensor.matmul(
                                        ps[:, par * 512 + lo_q:(par + 1) * 512],
                                        lhsT=kT[lo:hi, hp * T + kt * 128:
                                                hp * T + (kt + 1) * 128],
                                        rhs=qT[lo:hi, hp * T + qt * 512 + lo_q:
                                               hp * T + (qt + 1) * 512],
                                        start=True, stop=True,
                                    )
                                pt_t = ptp.tile([128, 1024], BF16, tag="pt")
                                ps3 = ps.rearrange("p (h q) -> p h q", q=512)
                                pt3 = pt_t.rearrange("p (h q) -> p h q", q=512)
                                nc.scalar.activation(
                                    pt3[:, :, lo_q:512], ps3[:, :, lo_q:512],
                                    EXP, bias=0.0, scale=0.125)
                                if i >= 0:
                                    # mask the diagonal 128x128 block (both
                                    # heads): q-subtile j == i
                                    blk = bass.AP(
                                        tensor=pt_t.tensor,
                                        offset=pt_t.offset + lo_q,
                                        ap=[list(pt_t.ap[0]), [512, 2], [1, 128]])
                                    mblk = bass.AP(
                                        tensor=maskt.tensor,
                                        offset=maskt.offset,
                                        ap=[list(maskt.ap[0]), [512, 2], [1, 128]])
                                    nc.vector.tensor_tensor(
                                        out=blk, in0=blk, in1=mblk, op=AX.mult)
                                pts[kt] = pt_t
                            fill(1 if is_last else per_slot)
                            if not is_last:
                                for kt in chunk:
                                    for j in range(4):
                                        if kt > 4 * qt + j:
                                            continue  # fully-dead block
                                        for par in range(2):
                                            av_mm(kt, j, par, pts[kt])
                            else:
                                # tail: finish bank 0 (q-subtiles 0,1) first
                                # so its normalize / attnT transpose /
                                # out-proj overlap bank 1's AV matmuls;
                                # leftover filler chains are drained right
                                # after each bank's transposes to cover the
                                # transpose latency before out-proj starts
                                for bank in range(2):
                                    for kt in chunk:
                                        for j in (2 * bank, 2 * bank + 1):
                                            if kt > 4 * qt + j:
                                                continue
                                            for par in range(2):
                                                av_mm(kt, j, par, pts[kt])
                                    normalize(hp, qt, avt, bank=bank)
                                    xpose_tt(4 * qt + 2 * bank)
                                    xpose_tt(4 * qt + 2 * bank + 1)
                                    fill(2 if bank == 0 else 3)
                        if not tail:
                            normalize(hp, qt, avt)

                    def normalize(hp, qt, avt, bank=None):
                        # rc[p, s] = 1 / denom(slot s); attn chunk cols
                        # (hp, par) of t-chunks 4qt..4qt+3 (or one bank's 2)
                        if bank is None:
                            rc = recp.tile([128, 8], F32, tag="rc")
                            nc.vector.reciprocal(
                                rc,
                                bass.AP(tensor=avt.tensor,
                                        offset=avt.offset + 64,
                                        ap=[list(avt.ap[0]), [512, 2],
                                            [130, 2], [65, 2]]))
                            jd, base = [512, 2], 0
                            rjd = [4, 2]
                        else:
                            rc = recp.tile([128, 4], F32, tag="rc")
                            nc.vector.reciprocal(
                                rc,
                                bass.AP(tensor=avt.tensor,
                                        offset=avt.offset + bank * 512 + 64,
                                        ap=[list(avt.ap[0]), [130, 2],
                                            [65, 2]]))
                            jd, base = [130, 2], bank * 512
                            rjd = [2, 2]
                        nj = 4 if bank is None else 2
                        j0 = 0 if bank is None else 2 * bank
                        for par in range(2):
                            if bank is None:
                                dstd = [[1024, 2], [512, 2]]
                                srcd = [[512, 2], [130, 2]]
                                rcd = [[4, 2], [2, 2]]
                            else:
                                dstd = [[512, 2]]
                                srcd = [[130, 2]]
                                rcd = [[2, 2]]
                            dst = bass.AP(
                                tensor=attn.tensor,
                                offset=attn.offset + (4 * qt + j0) * 512
                                + (hp * 2 + par) * 64,
                                ap=[list(attn.ap[0])] + dstd + [[1, 64]])
                            src = bass.AP(
                                tensor=avt.tensor,
                                offset=avt.offset + base + par * 65,
                                ap=[list(avt.ap[0])] + srcd + [[1, 64]])
                            rcb = bass.AP(
                                tensor=rc.tensor, offset=rc.offset + par,
                                ap=[list(rc.ap[0])] + rcd + [[0, 64]])
                            nc.vector.tensor_tensor(
                                out=dst, in0=src, in1=rcb, op=AX.mult)

                    def xpose_tt(tt):
                        nc.sync.dma_start_transpose(
                            out=bass.AP(
                                tensor=attnT.tensor,
                                offset=attnT.offset + tt * 128,
                                ap=[list(attnT.ap[0]), [T, 4], [1, 128]]),
                            in_=attn[:, tt * 512:(tt + 1) * 512])

                    def emit_attnT(qt):
                        for j in range(4):
                            xpose_tt(4 * qt + j)

                    def out_chain(mt, n, alt=False):
                        ps = pjpsum.tile([128, 512], F32, tag="pj", name="y")
                        for kk in range(4):
                            nc.tensor.matmul(
                                ps,
                                lhsT=attnT[:, kk * T + mt * 128:
                                           kk * T + (mt + 1) * 128],
                                rhs=woutb[:, kk * D + n * 512:
                                          kk * D + (n + 1) * 512],
                                start=(kk == 0), stop=(kk == 3))
                        yt = yo.tile([128, 512], F32, tag="yt", name="yt")
                        nc.vector.tensor_copy(yt, ps)
                        # final group: issue from the (idle) ACT queue so the
                        # SP sequencer's issue rate doesn't pace the drain
                        q = nc.scalar if alt else nc.sync
                        q.dma_start(
                            out=out_ext[mt * 128:(mt + 1) * 128,
                                        n * 512:(n + 1) * 512], in_=yt)

                    def emit_outproj(mts, alt=False):
                        for mt in mts:
                            for n in range(2):
                                out_chain(mt, n, alt=alt)

                    from collections import deque

                    # Projection chains for head-pair hp+1 (and, on the last
                    # row, out-proj chains) are drained as fillers between an
                    # attention chunk's exp and its AV matmuls, so the PE has
                    # exp-independent work exactly where it would stall.
                    for hp in range(HLOC // 2):
                        fillers = deque()
                        if hp < 3:
                            for sec in (0, 1):
                                for n in range(NQT):
                                    fillers.append(
                                        (lambda m, s, nn:
                                         lambda: qk_chain(m, s, nn))(
                                             hp + 1, sec, n))
                        for qt in range(NQT):
                            if hp == 0:
                                if qt == 0:
                                    emit_vproj(range(0, 4))
                                    emit_qkproj(0, ns=[0])
                                else:
                                    emit_qkproj(0, ns=[qt])
                                    emit_vproj(range(4 * qt, 4 * qt + 4))
                            if hp == 3 and qt >= 1:
                                for mt in range(4 * (qt - 1), 4 * qt):
                                    for n in range(2):
                                        fillers.append(
                                            (lambda m, nn:
                                             lambda: out_chain(m, nn))(mt, n))
                            tail = hp == 3 and qt == 3
                            emit_unit(hp, qt, fillers,
                                      per_slot=(2 if hp == 3 else
                                                2 if qt >= 2 else 0),
                                      tail=tail)
                            if hp == 3 and not tail:
                                emit_attnT(qt)
                        if hp < 3:
                            while fillers:
                                fillers.popleft()()
                    emit_outproj(range(12, 16), alt=True)
                    while fillers:
                        fillers.popleft()()
                    if DEBUG_DUMP:
                        with tc.tile_pool(name="dbgp", bufs=2) as dbgp:
                            for nm, src in (("xT", xT), ("qT", qT), ("kT", kT),
                                            ("vaug", vaug), ("attn", attn),
                                            ("attnT", attnT)):
                                w = src.shape[1]
                                for c0 in range(0, w, 2048):
                                    cw = min(2048, w - c0)
                                    t = dbgp.tile([128, cw], F32, tag="dbg")
                                    nc.vector.tensor_copy(t, src[:, c0:c0 + cw])
                                    nc.sync.dma_start(
                                        out=dbg[nm][0:128, c0:c0 + cw], in_=t)

    _split_multiwaits(nc)
    return nc


_NC_CACHE = {}


def get_nc():
    if "nc" not in _NC_CACHE:
        _NC_CACHE["nc"] = build_nc()
    return _NC_CACHE["nc"]


def make_in_maps(x, W_qkv, b_qkv, W_out):
    in_maps = []
    for c in range(8):
        b, g = c // 2, c % 2
        s = slice(512 * g, 512 * (g + 1))
        wslice = np.concatenate(
            [W_qkv[:, 512 * g:512 * (g + 1)],
             W_qkv[:, 1024 + 512 * g:1024 + 512 * (g + 1)],
             W_qkv[:, 2048 + 512 * g:2048 + 512 * (g + 1)]], axis=1)
        bslice = np.concatenate(
            [b_qkv[512 * g:512 * (g + 1)],
             b_qkv[1024 + 512 * g:1024 + 512 * (g + 1)],
             b_qkv[2048 + 512 * g:2048 + 512 * (g + 1)]])
        in_maps.append({
            "x": np.ascontiguousarray(x[b], dtype=np.float32),
            "W_qkv": np.ascontiguousarray(wslice, dtype=np.float32),
            "b_qkv": np.ascontiguousarray(bslice, dtype=np.float32),
            "W_out": np.ascontiguousarray(W_out[s], dtype=np.float32),
        })
    return in_maps


def kernel(x, W_qkv, b_qkv, W_out, b_out):
    x = np.asarray(x)
    W_qkv = np.asarray(W_qkv)
    b_qkv = np.asarray(b_qkv)
    W_out = np.asarray(W_out)
    b_out = np.asarray(b_out)
    nc = get_nc()
    in_maps = make_in_maps(x, W_qkv, b_qkv, W_out)
    res = run_bass_kernel_spmd(nc, in_maps, core_ids=list(range(8))).results
    out = np.stack(
        [res[2 * b]["out"] + res[2 * b + 1]["out"] for b in range(4)], axis=0)
    out = out + b_out[None, None, :]
    return out.astype(np.float32)
